# revision 18
# baseline (speedup 1.0000x reference)
"""Trainium2 Bass kernel for the CurriculumDMGHANmae model.

Data-parallel over batch across 8 NeuronCores (256 samples/core). Per core,
the whole network runs in a feature-major layout (features on partitions,
batch*seq on the free dim) so every projection is a PE matmul with no
activations transposed except the initial input transpose.

The Mamba selective scan uses the DVE tensor_tensor_scan instruction: with
partitions = (d, n) state pairs and free = (sample, seqpos), one scan
instruction per 128-partition tile computes the whole recurrence; zeroing
the multiplier at every l==0 column makes the cross-sample scan exact.

Matmuls run in float32r (full-rate PE fp32, ~2^-13 input rounding); the
scan itself and all elementwise math stay in fp32.
"""
import sys

sys.path.insert(0, "/opt/trn_rl_repo")

import numpy as np
import orjson

import concourse.bass as bass
import concourse.bass2jax as bass2jax
import concourse.bass_utils as bass_utils
import concourse.mybir as mybir
import concourse.tile as tile
import bass_rust

F32 = mybir.dt.float32
F32R = mybir.dt.float32r
AF = mybir.ActivationFunctionType
ALU = mybir.AluOpType
AX = mybir.AxisListType

# ---------------------------------------------------------------------------
# Toolchain workarounds (this walrus build rejects >1 sync wait per
# instruction, and the Tile exit drain carries one wait per logical
# processor). Hoist excess waits onto same-engine NoOps at the BIR level.
# ---------------------------------------------------------------------------
_MAXW = 1
_ctr = [0]


def _split_waits_json(bir_bytes):
    d = orjson.loads(bir_bytes)
    changed = False
    for fn in d.get("functions", []):
        for blk in fn.get("blocks", []):
            out = []
            for ins in blk.get("instructions", []):
                si = ins.get("sync_info")
                waits = si.get("on_wait") if si else None
                if waits and len(waits) > _MAXW:
                    extra = waits[:-_MAXW]
                    si["on_wait"] = waits[-_MAXW:]
                    for i in range(0, len(extra), _MAXW):
                        _ctr[0] += 1
                        out.append(
                            {
                                "engine": ins["engine"],
                                "ins": [],
                                "name": f"I-waitsplit-{_ctr[0]}",
                                "opcode": "NoOp",
                                "outs": [],
                                "sync_info": {
                                    "on_update": [],
                                    "on_wait": extra[i : i + _MAXW],
                                },
                            }
                        )
                    changed = True
                out.append(ins)
            blk["instructions"] = out
    return orjson.dumps(d) if changed else bir_bytes


_orig_compile_bir_kernel = bass_utils.compile_bir_kernel


def _patched_compile_bir_kernel(bir_json, tmpdir, neff_name="file.neff"):
    if isinstance(bir_json, str):
        bir_json = bir_json.encode()
    return _orig_compile_bir_kernel(
        _split_waits_json(bir_json), tmpdir, neff_name=neff_name
    )


def _patched_drain_and_barrier(self, tick_clock, wait_clock):
    nc = self.nc
    probe = nc.sync.nop()
    wait_clock.add_sem_waits(
        probe.ins, tile.ScopedClock({None: tick_clock.global_clock})
    )
    si = probe.ins.sync_info
    waits = list(si.on_wait) if si is not None else []
    if len(waits) > 1:
        probe.ins.sync_info = bass_rust.SyncInfo(on_wait=waits[:1], on_update=[])
        for w in waits[1:]:
            extra = nc.sync.nop()
            extra.ins.sync_info = bass_rust.SyncInfo(on_wait=[w], on_update=[])
    nc.sync.drain()
    nc.all_engine_barrier()
    assert self.sems is not None
    popped = nc._tile_sem_poison_stack.pop()
    assert popped is self._sem_poison
    nc.clear_and_free_semaphores(list(self.sems.allocated().values()))
    nc.all_engine_barrier()


def _apply_patches():
    bass_utils.compile_bir_kernel = _patched_compile_bir_kernel
    bass2jax.compile_bir_kernel = _patched_compile_bir_kernel
    tile.TileContext._drain_and_barrier = _patched_drain_and_barrier


_apply_patches()

# ---------------------------------------------------------------------------
# Model constants
# ---------------------------------------------------------------------------
NCORES = 8
B_FULL = 2048
BC = B_FULL // NCORES  # 256 samples per core
L = 12
DM = 256
DI = 512
DS = 16
DTR = 16
NCLS = (5, 30, 80, 200, 600, 1500)
LN_EPS = 1e-5
NQ = 4  # quarters per core
QB = BC // NQ  # 64 samples per quarter
COLS = QB * L  # 768 free columns per quarter
HCH = 384  # psum chunk (half of COLS)


def _cdiv(a, b):
    return (a + b - 1) // b


def _host_prep(w):
    """Build all device-side constant arrays from the raw model weights."""
    f32 = lambda x: np.ascontiguousarray(x, dtype=np.float32)
    WfL = w["fusion_w"][:, :DM]
    WfR = w["fusion_w"][:, DM:]
    out = {}
    out["w1f_T"] = f32((WfL @ w["coi_proj_w"]).T)  # [768, 256]
    out["w2f_T"] = f32((WfR @ w["coi_mae_proj_w"]).T)  # [768, 256]
    bias_fused = WfL @ w["coi_proj_b"] + WfR @ w["coi_mae_proj_b"] + w["fusion_b"]
    out["bias_fused"] = f32(bias_fused.reshape(2, 128).T)  # [128, 2]
    out["ln_g"] = f32(w["ln_g"].reshape(2, 128).T)
    out["ln_b"] = f32(w["ln_b"].reshape(2, 128).T)
    # Wbig: conv folded into the xc half of in_proj.
    Win_x = w["in_proj_w"][:DI]  # [512, 256]
    Win_z = w["in_proj_w"][DI:]  # [512, 256]
    wbig = np.zeros((4 * DM, DI), dtype=np.float32)
    for k in range(4):
        wbig[k * DM : (k + 1) * DM, :] = (w["conv_w"][:, k][:, None] * Win_x).T
    out["wbig_T"] = f32(wbig)  # [1024, 512]
    out["wz_T"] = f32(Win_z.T)  # [256, 512]
    out["conv_b_neg"] = f32(-w["conv_b"].reshape(4, 128).T)  # [128, 4]
    out["conv_b"] = f32(w["conv_b"].reshape(4, 128).T)
    out["x_proj_T"] = f32(w["x_proj_w"].T)  # [512, 48]
    out["dt_proj_T"] = f32(w["dt_proj_w"].T)  # [16, 512]
    out["dt_proj_b"] = f32(w["dt_proj_b"].reshape(4, 128).T)  # [128, 4]
    A = -np.exp(w["A_log"])  # [512, 16]
    acols = np.zeros((128, 64), dtype=np.float32)
    for j in range(64):
        for p in range(128):
            acols[p, j] = A[8 * j + p // 16, p % 16]
    out["a_cols"] = f32(acols)
    out["dp_col"] = f32(w["Dp"].reshape(4, 128).T)  # [128, 4]
    out["out_proj_T"] = f32(w["out_proj_w"].T / float(L))  # [512, 256], pool fold
    selrep = np.zeros((128, 1024), dtype=np.float32)
    for a in range(2):
        for m in range(8):
            for p in range(128):
                selrep[64 * a + 8 * m + p // 16, 128 * m + p] = 1.0
    out["selrep"] = selrep  # [128,1024]: same [64,128] pattern at bases 0/64
    sel16b = np.zeros((48, 128), dtype=np.float32)
    for p in range(128):
        sel16b[16 + p % 16, p] = 1.0
    out["sel16b"] = sel16b  # vs dbcT[0:48]: picks rows 16..31 (B)
    sel16c = np.zeros((48, 128), dtype=np.float32)
    for p in range(128):
        sel16c[32 + p % 16, p] = 1.0
    out["sel16c"] = sel16c  # vs dbcT[0:48]: picks rows 32..47 (C)
    selred = np.zeros((128, 16 * 128), dtype=np.float32)
    for jj in range(16):
        for p in range(128):
            selred[p, 128 * jj + 8 * jj + p // 16] = 1.0
    out["selred"] = f32(selred)  # [128, 2048], slice jj = [:, 128jj:128jj+128]
    out["ones_red"] = f32(np.full((128, 1), 1.0 / DM, dtype=np.float32))
    out["ones_row"] = f32(np.ones((1, 128), dtype=np.float32))
    out["eps_col"] = f32(np.full((128, 1), LN_EPS, dtype=np.float32))
    out["ident"] = f32(np.eye(128, dtype=np.float32))
    # head
    gate_T = np.zeros((DI, 5 * DM), dtype=np.float32)
    for i in range(5):
        gate_T[:, DM * i : DM * (i + 1)] = w["gate_w"][i].T
    out["gate_T"] = f32(gate_T)
    out["gate_b_neg"] = f32(
        -np.stack([w["gate_b"][i].reshape(2, 128).T for i in range(5)], 0)
        .transpose(1, 0, 2)
        .reshape(128, 10)
    )  # col 2i+m
    attn_v_T = np.zeros((DM, 6 * DM), dtype=np.float32)
    attn_o_T = np.zeros((DM, 6 * DM), dtype=np.float32)
    attn_vb = np.zeros((128, 12), dtype=np.float32)
    attn_ob = np.zeros((128, 12), dtype=np.float32)
    for i in range(6):
        wv = w["attn_in_w"][i][2 * DM :]  # [256, 256]
        bv = w["attn_in_b"][i][2 * DM :]
        attn_v_T[:, DM * i : DM * (i + 1)] = wv.T
        attn_o_T[:, DM * i : DM * (i + 1)] = w["attn_out_w"][i].T
        attn_vb[:, 2 * i : 2 * i + 2] = bv.reshape(2, 128).T
        attn_ob[:, 2 * i : 2 * i + 2] = w["attn_out_b"][i].reshape(2, 128).T
    out["attn_v_T"] = f32(attn_v_T)
    out["attn_o_T"] = f32(attn_o_T)
    out["attn_vb"] = f32(attn_vb)
    out["attn_ob"] = f32(attn_ob)
    cls_T = np.zeros((DM, sum(NCLS)), dtype=np.float32)
    off = 0
    cls_chunks = []  # (level, m_off_global, m_size, bias_col)
    cls_b = np.zeros((128, 22), dtype=np.float32)
    col = 0
    for i, n in enumerate(NCLS):
        cls_T[:, off : off + n] = w["cls_w"][i].T
        for mc in range(_cdiv(n, 128)):
            sz = min(128, n - 128 * mc)
            cls_b[:sz, col] = w["cls_b"][i][128 * mc : 128 * mc + sz]
            cls_chunks.append((i, off + 128 * mc, sz, col))
            col += 1
        off += n
    out["cls_T"] = f32(cls_T)
    out["cls_b"] = f32(cls_b)
    return out, cls_chunks


# ---------------------------------------------------------------------------
# Program builder
# ---------------------------------------------------------------------------
def build_program(cls_chunks):
    nc = bass.Bass("TRN2", debug=False, num_devices=NCORES)

    din = {}

    def dram_in(name, shape, dt=F32R):
        din[name] = nc.dram_tensor(name, list(shape), dt, kind="ExternalInput").ap()
        return din[name]

    d_coi = dram_in("coi", (BC, L, 768)).rearrange("b l f -> (b l) f")
    d_mae = dram_in("coi_mae", (BC, 768))
    d_w1f = dram_in("w1f_T", (768, 256))
    d_w2f = dram_in("w2f_T", (768, 256))
    d_biasf = dram_in("bias_fused", (128, 2), F32)
    d_lng = dram_in("ln_g", (128, 2), F32)
    d_lnb = dram_in("ln_b", (128, 2), F32)
    d_wbig = dram_in("wbig_T", (1024, 512))
    d_wz = dram_in("wz_T", (256, 512))
    d_convbn = dram_in("conv_b_neg", (128, 4), F32)
    d_convb = dram_in("conv_b", (128, 4), F32)
    d_xproj = dram_in("x_proj_T", (512, 48))
    d_dtproj = dram_in("dt_proj_T", (16, 512))
    d_dtb = dram_in("dt_proj_b", (128, 4), F32)
    d_acols = dram_in("a_cols", (128, 64), F32)
    d_dp = dram_in("dp_col", (128, 4), F32)
    d_outproj = dram_in("out_proj_T", (512, 256))
    d_selrep = dram_in("selrep", (128, 1024))
    d_sel16b = dram_in("sel16b", (48, 128))
    d_sel16c = dram_in("sel16c", (48, 128))
    d_selred = dram_in("selred", (128, 2048))
    d_onesred = dram_in("ones_red", (128, 1))
    d_onesrow = dram_in("ones_row", (1, 128))
    d_epscol = dram_in("eps_col", (128, 1), F32)
    d_ident = dram_in("ident", (128, 128))
    d_gate = dram_in("gate_T", (512, 1280))
    d_gbn = dram_in("gate_b_neg", (128, 10), F32)
    d_avT = dram_in("attn_v_T", (256, 1536))
    d_aoT = dram_in("attn_o_T", (256, 1536))
    d_avb = dram_in("attn_vb", (128, 12), F32)
    d_aob = dram_in("attn_ob", (128, 12), F32)
    d_clsT = dram_in("cls_T", (256, 2415))
    d_clsb = dram_in("cls_b", (128, 22), F32)

    d_out = [
        nc.dram_tensor(f"out{i}", [BC, n], F32, kind="ExternalOutput").ap()
        for i, n in enumerate(NCLS)
    ]

    with tile.TileContext(nc) as tc:
        from contextlib import ExitStack

        with ExitStack() as ctx:
            cp = ctx.enter_context(tc.tile_pool(name="consts", bufs=1))
            pp = ctx.enter_context(tc.tile_pool(name="persist", bufs=1))

            def load(pool, d, shape, dt=F32R, tag=None):
                t_name = tag or f"c{len(din)}_{id(d)}"
                t = pool.tile(list(shape), dt, tag=t_name, name=t_name)
                nc.sync.dma_start(t[:, :], d)
                return t

            # --- constants ---
            t_w1f = [
                load(cp, d_w1f[128 * k : 128 * (k + 1), :], (128, 256), tag=f"w1f{k}")
                for k in range(6)
            ]
            t_w2f = [
                load(cp, d_w2f[128 * k : 128 * (k + 1), :], (128, 256), tag=f"w2f{k}")
                for k in range(6)
            ]
            t_biasf = load(cp, d_biasf, (128, 2), F32, tag="biasf")
            t_lng = load(cp, d_lng, (128, 2), F32, tag="lng")
            t_lnb = load(cp, d_lnb, (128, 2), F32, tag="lnb")
            t_wbig = [
                load(cp, d_wbig[128 * k : 128 * (k + 1), :], (128, 512), tag=f"wbig{k}")
                for k in range(8)
            ]
            t_wz = [
                load(cp, d_wz[128 * k : 128 * (k + 1), :], (128, 512), tag=f"wz{k}")
                for k in range(2)
            ]
            t_convbn = load(cp, d_convbn, (128, 4), F32, tag="convbn")
            t_convb = load(cp, d_convb, (128, 4), F32, tag="convb")
            t_xproj = [
                load(cp, d_xproj[128 * k : 128 * (k + 1), :], (128, 48), tag=f"xp{k}")
                for k in range(4)
            ]
            t_dtproj = load(cp, d_dtproj, (16, 512), tag="dtp")
            t_dtb = load(cp, d_dtb, (128, 4), F32, tag="dtb")
            t_acols = load(cp, d_acols, (128, 64), F32, tag="acols")
            t_dp = load(cp, d_dp, (128, 4), F32, tag="dp")
            t_outproj = [
                load(cp, d_outproj[128 * k : 128 * (k + 1), :], (128, 256), tag=f"op{k}")
                for k in range(4)
            ]
            t_selrep = load(cp, d_selrep, (128, 1024), tag="selrep")
            t_sel16b = load(cp, d_sel16b, (48, 128), tag="sel16b")
            t_sel16c = load(cp, d_sel16c, (48, 128), tag="sel16c")
            t_selred = load(cp, d_selred, (128, 2048), tag="selred")
            t_onesred = load(cp, d_onesred, (128, 1), tag="onesred")
            t_onesrow = load(cp, d_onesrow, (1, 128), tag="onesrow")
            t_epscol = load(cp, d_epscol, (128, 1), F32, tag="epscol")
            t_ident = load(cp, d_ident, (128, 128), tag="ident")

            # persistent across quarters
            t_pooled = [pp.tile([128, BC], F32R, tag=f"pooled{m}", name=f"pooled{m}") for m in range(2)]

            ps_mm = ctx.enter_context(
                tc.tile_pool(name="ps_mm", bufs=2, space="PSUM")
            )
            ps_tr = ps_mm
            ps_strip = ps_mm

            # --- stage 0: mae path ---
            with tc.tile_pool(name="mae", bufs=1) as mp:
                rows = [
                    load(mp, d_mae[128 * r : 128 * (r + 1), :], (128, 768), tag=f"mr{r}")
                    for r in range(2)
                ]
                maeT = [mp.tile([128, 256], F32R, tag=f"maeT{f}", name=f"maeT{f}") for f in range(6)]
                for f in range(6):
                    p = ps_tr.tile([128, 256], F32R, tag="mm", name="mm")
                    for r in range(2):
                        nc.tensor.transpose(
                            p[:, 128 * r : 128 * (r + 1)],
                            rows[r][:, 128 * f : 128 * (f + 1)],
                            t_ident[:, :],
                        )
                    nc.scalar.copy(maeT[f][:, :], p[:, :].bitcast(F32))
                t_maef = [
                    pp.tile([128, BC], F32, tag=f"maef{m}", name=f"maef{m}") for m in range(2)
                ]
                for m in range(2):
                    p = ps_mm.tile([128, 384], F32, tag="mm", name="mm")
                    for k in range(6):
                        nc.tensor.matmul(
                            p[:, 0:256],
                            t_w2f[k][:, 128 * m : 128 * (m + 1)],
                            maeT[k][:, :],
                            start=(k == 0),
                            stop=(k == 5),
                        )
                    nc.scalar.activation(
                        t_maef[m][:, :], p[:, 0:256], AF.Identity,
                        bias=t_biasf[:, m : m + 1],
                    )

            # --- quarters ---
            qctx = ExitStack()
            wp = qctx.enter_context(tc.tile_pool(name="work", bufs=1))
            wp2 = qctx.enter_context(tc.tile_pool(name="work2", bufs=2))
            sp = qctx.enter_context(tc.tile_pool(name="scan", bufs=2))
            ps_a = qctx.enter_context(tc.tile_pool(name="ps_a", bufs=2, space="PSUM"))
            ps_y = qctx.enter_context(tc.tile_pool(name="ps_y", bufs=1, space="PSUM"))

            for q in range(NQ):
                # Phase A: load + transpose coi
                coiT = [wp2.tile([128, COLS], F32R, tag=f"coiT{f}", name=f"coiT{f}", bufs=1) for f in range(6)]
                for r in range(6):
                    rt = wp2.tile([128, 768], F32R, tag="coirow", name="coirow", bufs=1)
                    # rows of this tile are (l, b) l-major: 2 l values x 64 b
                    for i in range(2):
                        nc.sync.dma_start(
                            rt[64 * i : 64 * (i + 1), :],
                            d_coi.rearrange("(b l) f -> b l f", l=L)[
                                QB * q : QB * (q + 1), 2 * r + i, :
                            ],
                        )
                    for fp in range(3):
                        p = ps_tr.tile([128, 256], F32R, tag="mm", name="mm")
                        for f2 in range(2):
                            f = 2 * fp + f2
                            nc.tensor.transpose(
                                p[:, 128 * f2 : 128 * (f2 + 1)],
                                rt[:, 128 * f : 128 * (f + 1)],
                                t_ident[:, :],
                            )
                        for f2 in range(2):
                            f = 2 * fp + f2
                            nc.scalar.copy(
                                coiT[f][:, 128 * r : 128 * (r + 1)],
                                p[:, 128 * f2 : 128 * (f2 + 1)].bitcast(F32),
                            )

                # Phase B: fused projection + mae + bias + relu
                fusedT = [wp.tile([128, COLS], F32R, tag=f"fusedT{m}", name=f"fusedT{m}") for m in range(2)]
                for m in range(2):
                    for c in range(2):
                        p = ps_mm.tile([128, HCH], F32, tag="mm", name="mm")
                        for k in range(6):
                            nc.tensor.matmul(
                                p[:, :],
                                t_w1f[k][:, 128 * m : 128 * (m + 1)],
                                coiT[k][:, HCH * c : HCH * (c + 1)],
                                start=(k == 0),
                                stop=(k == 5),
                            )
                        tmp = wp2.tile([128, HCH], F32, tag="y1", name="btmp", bufs=1)
                        nc.vector.scalar_tensor_tensor(
                            out=tmp[:, :].rearrange("p (l b) -> p b l", b=QB),
                            in0=p[:, :].rearrange("p (l b) -> p b l", b=QB),
                            scalar=t_biasf[:, m : m + 1],
                            in1=t_maef[m][:, QB * q : QB * (q + 1)]
                            .broadcast_to([128, QB, 6]),
                            op0=ALU.add,
                            op1=ALU.add,
                        )
                        nc.scalar.activation(
                            fusedT[m][:, HCH * c : HCH * (c + 1)], tmp[:, :], AF.Relu
                        )

                # Phase C: LayerNorm
                lnT = [wp2.tile([128, COLS], F32R, tag=f"coiT{m}", name=f"lnT{m}", bufs=1) for m in range(2)]
                sqT = [wp2.tile([128, COLS], F32R, tag=f"sqT{m}", name=f"sqT{m}", bufs=1) for m in range(2)]
                for m in range(2):
                    nc.scalar.activation(sqT[m][:, :], fusedT[m][:, :].bitcast(F32), AF.Square)
                t2T = [wp2.tile([128, COLS], F32, tag=f"t2T{m}", name=f"t2T{m}", bufs=1) for m in range(2)]
                reps = []
                for c in range(2):
                    pmean = ps_strip.tile([1, HCH], F32, tag="mm", name="pmean")
                    pex2 = ps_strip.tile([1, HCH], F32, tag="mm", name="pex2")
                    for m in range(2):
                        nc.tensor.matmul(
                            pmean[:, :], t_onesred[:, :],
                            fusedT[m][:, HCH * c : HCH * (c + 1)],
                            start=(m == 0), stop=(m == 1),
                        )
                        nc.tensor.matmul(
                            pex2[:, :], t_onesred[:, :],
                            sqT[m][:, HCH * c : HCH * (c + 1)],
                            start=(m == 0), stop=(m == 1),
                        )
                    mean_sb = wp2.tile([1, HCH], F32R, tag="mean_sb", name="mean_sb", bufs=2)
                    nc.scalar.copy(mean_sb[:, :], pmean[:, :])
                    msq = wp2.tile([1, HCH], F32, tag="msq", name="msq")
                    nc.vector.tensor_tensor(
                        out=msq[:, :], in0=mean_sb[:, :].bitcast(F32),
                        in1=mean_sb[:, :].bitcast(F32), op=ALU.mult,
                    )
                    var = wp2.tile([1, HCH], F32, tag="var", name="var")
                    nc.vector.tensor_tensor(
                        out=var[:, :], in0=pex2[:, :], in1=msq[:, :], op=ALU.subtract,
                    )
                    lnv = wp2.tile([1, HCH], F32, tag="lnv", name="lnv")
                    nc.scalar.activation(
                        lnv[:, :], var[:, :], AF.Ln, bias=t_epscol[0:1, :]
                    )
                    rstd = wp2.tile([1, HCH], F32R, tag="rstd", name="rstd", bufs=2)
                    nc.scalar.activation(rstd[:, :], lnv[:, :], AF.Exp, scale=-0.5)
                    reps.append((mean_sb, rstd))
                for c in range(2):
                    mean_sb, rstd = reps[c]
                    prep_m = ps_strip.tile([128, HCH], F32, tag="mm", name="prepm")
                    nc.tensor.matmul(
                        prep_m[:, :], t_onesrow[:, :], mean_sb[:, :],
                        start=True, stop=True,
                    )
                    prep_r = ps_strip.tile([128, HCH], F32, tag="mm", name="prepr")
                    nc.tensor.matmul(
                        prep_r[:, :], t_onesrow[:, :], rstd[:, :],
                        start=True, stop=True,
                    )
                    for m in range(2):
                        t1 = wp2.tile([128, HCH], F32, tag="lnt1", name="lnt1", bufs=1)
                        nc.vector.tensor_tensor(
                            out=t1[:, :],
                            in0=fusedT[m][:, HCH * c : HCH * (c + 1)].bitcast(F32),
                            in1=prep_m[:, :], op=ALU.subtract,
                        )
                        nc.vector.tensor_tensor(
                            out=t2T[m][:, HCH * c : HCH * (c + 1)],
                            in0=t1[:, :], in1=prep_r[:, :], op=ALU.mult,
                        )
                for m in range(2):
                    nc.scalar.activation(
                        lnT[m][:, :], t2T[m][:, :], AF.Identity,
                        bias=t_lnb[:, m : m + 1], scale=t_lng[:, m : m + 1],
                    )

                # Phase D: in_proj with folded conv (xc) and z + silu
                xcT = [wp.tile([128, COLS], F32R, tag=f"xcT{g}", name=f"xcT{g}") for g in range(4)]
                szT = [wp.tile([128, COLS], F32, tag=f"szT{g}", name=f"szT{g}") for g in range(4)]
                for g in range(4):
                    for c in range(2):
                        p = ps_mm.tile([128, HCH], F32, tag="mm", name="mm")
                        # shift k=3 (offset 0), full range, starts accumulation
                        for h in range(2):
                            nc.tensor.matmul(
                                p[:, :],
                                t_wbig[6 + h][:, 128 * g : 128 * (g + 1)],
                                lnT[h][:, HCH * c : HCH * (c + 1)],
                                start=(h == 0), stop=False,
                                skip_group_check=True,
                            )
                        for k in (2, 1, 0):
                            o = 3 - k
                            for h in range(2):
                                if c == 0:
                                    outap = p[:, QB * o : HCH]
                                    rhsap = lnT[h][:, 0 : HCH - QB * o]
                                else:
                                    outap = p[:, :]
                                    rhsap = lnT[h][:, HCH - QB * o : 2 * HCH - QB * o]
                                nc.tensor.matmul(
                                    outap,
                                    t_wbig[2 * k + h][:, 128 * g : 128 * (g + 1)],
                                    rhsap,
                                    start=False, stop=(k == 0 and h == 1),
                                    skip_group_check=True,
                                )
                        # silu evac: xc = (p + b) * sigmoid(p + b)
                        e = wp2.tile([128, HCH], F32, tag="se", name="se", bufs=1)
                        nc.scalar.activation(
                            e[:, :], p[:, :], AF.Exp, scale=-1.0,
                            bias=t_convbn[:, g : g + 1],
                        )
                        f1 = wp2.tile([128, HCH], F32, tag="sf", name="sf", bufs=1)
                        nc.vector.tensor_scalar(
                            out=f1[:, :], in0=e[:, :], scalar1=1.0, scalar2=None,
                            op0=ALU.add,
                        )
                        r1 = wp2.tile([128, HCH], F32, tag="sr", name="sr", bufs=1)
                        nc.vector.reciprocal(r1[:, :], f1[:, :])
                        nc.vector.scalar_tensor_tensor(
                            out=xcT[g][:, HCH * c : HCH * (c + 1)],
                            in0=p[:, :], scalar=t_convb[:, g : g + 1],
                            in1=r1[:, :], op0=ALU.add, op1=ALU.mult,
                        )
                for g in range(4):
                    for c in range(2):
                        p = ps_mm.tile([128, HCH], F32, tag="mm", name="mm")
                        for h in range(2):
                            nc.tensor.matmul(
                                p[:, :],
                                t_wz[h][:, 128 * g : 128 * (g + 1)],
                                lnT[h][:, HCH * c : HCH * (c + 1)],
                                start=(h == 0), stop=(h == 1),
                            )
                        e = wp2.tile([128, HCH], F32, tag="se", name="se", bufs=1)
                        nc.scalar.activation(e[:, :], p[:, :], AF.Exp, scale=-1.0)
                        f1 = wp2.tile([128, HCH], F32, tag="sf", name="sf", bufs=1)
                        nc.vector.tensor_scalar(
                            out=f1[:, :], in0=e[:, :], scalar1=1.0, scalar2=None,
                            op0=ALU.add,
                        )
                        r1 = wp2.tile([128, HCH], F32, tag="sr", name="sr", bufs=1)
                        nc.vector.reciprocal(r1[:, :], f1[:, :])
                        nc.vector.scalar_tensor_tensor(
                            out=szT[g][:, HCH * c : HCH * (c + 1)],
                            in0=p[:, :], scalar=0.0,
                            in1=r1[:, :], op0=ALU.bypass, op1=ALU.mult,
                        )

                # Phase E: x_proj -> dbc; dt; u; B/C replication
                dbcT = wp.tile([48, COLS], F32R, tag="dbcT", name="dbcT")
                for c in range(2):
                    p = ps_mm.tile([128, HCH], F32, tag="mm", name="mm")
                    for k in range(4):
                        nc.tensor.matmul(
                            p[0:48, :], t_xproj[k][:, :],
                            xcT[k][:, HCH * c : HCH * (c + 1)],
                            start=(k == 0), stop=(k == 3),
                        )
                    nc.scalar.copy(
                        dbcT[:, HCH * c : HCH * (c + 1)], p[0:48, :]
                    )
                dtT = [wp.tile([128, COLS], F32R, tag=f"dtT{g}", name=f"dtT{g}") for g in range(4)]
                for g in range(4):
                    for c in range(2):
                        p = ps_mm.tile([128, HCH], F32, tag="mm", name="mm")
                        nc.tensor.matmul(
                            p[:, :], t_dtproj[:, 128 * g : 128 * (g + 1)],
                            dbcT[0:16, HCH * c : HCH * (c + 1)],
                            start=True, stop=True,
                        )
                        e = wp2.tile([128, HCH], F32, tag="spe", name="spe", bufs=1)
                        nc.scalar.activation(
                            e[:, :], p[:, :], AF.Exp, bias=t_dtb[:, g : g + 1]
                        )
                        nc.scalar.activation(
                            dtT[g][:, HCH * c : HCH * (c + 1)], e[:, :], AF.Ln,
                            bias=1.0,
                        )
                uT = [wp.tile([128, COLS], F32R, tag=f"uT{g}", name=f"uT{g}") for g in range(4)]
                for g in range(4):
                    nc.vector.tensor_tensor(
                        out=uT[g][:, :], in0=dtT[g][:, :].bitcast(F32),
                        in1=xcT[g][:, :].bitcast(F32), op=ALU.mult,
                    )
                t_brep = wp.tile([128, COLS], F32, tag="brep", name="brep")
                t_crep = wp.tile([128, COLS], F32, tag="crep", name="crep")
                for c in range(2):
                    pb = ps_mm.tile([128, HCH], F32, tag="mm", name="mm")
                    nc.tensor.matmul(
                        pb[:, :], t_sel16b[:, :], dbcT[0:48, HCH * c : HCH * (c + 1)],
                        start=True, stop=True,
                    )
                    nc.scalar.activation(
                        t_brep[:, :].rearrange("p (b l) -> p b l", l=L)[
                            :, :, 6 * c : 6 * (c + 1)
                        ],
                        pb[:, :].rearrange("p (l b) -> p b l", b=QB),
                        AF.Copy,
                    )
                    pc = ps_mm.tile([128, HCH], F32, tag="mm", name="mm")
                    nc.tensor.matmul(
                        pc[:, :], t_sel16c[:, :], dbcT[0:48, HCH * c : HCH * (c + 1)],
                        start=True, stop=True,
                    )
                    nc.scalar.activation(
                        t_crep[:, :].rearrange("p (b l) -> p b l", l=L)[
                            :, :, 6 * c : 6 * (c + 1)
                        ],
                        pc[:, :].rearrange("p (l b) -> p b l", b=QB),
                        AF.Copy,
                    )

                # Phase F: scan core over 64 dn-tiles
                y2T = [wp.tile([128, COLS], F32R, tag=(f"fusedT{g}" if g < 2 else f"y2T{g}"), name=f"y2T{g}") for g in range(4)]
                for g in range(4):
                    py = ps_y.tile([128, COLS], F32, tag="py", name="py")
                    for jj in range(16):
                        j = 16 * g + jj
                        prow = 8 * jj
                        ja, jm = jj // 8, jj % 8
                        lsel = t_selrep[64 * ja : 64 * (ja + 1), 128 * jm : 128 * (jm + 1)]
                        pa = ps_a.tile([128, COLS], F32, tag="pab", name="pab")
                        nc.tensor.matmul(
                            pa[:, 0:512], lsel,
                            dtT[g][64 * ja : 64 * (ja + 1), 0:512],
                            start=True, stop=True,
                        )
                        nc.tensor.matmul(
                            pa[:, 512:768], lsel,
                            dtT[g][64 * ja : 64 * (ja + 1), 512:768],
                            start=True, stop=True,
                        )
                        dA = sp.tile([128, COLS], F32, tag="dA", name="dA")
                        nc.scalar.activation(
                            dA[:, :].rearrange("p (b l) -> p b l", l=L),
                            pa[:, :].rearrange("p (l b) -> p b l", b=QB),
                            AF.Exp, scale=t_acols[:, j : j + 1],
                        )
                        nc.gpsimd.memset(
                            dA[:, :].rearrange("p (b l) -> p b l", l=L)[:, :, 0:1], 0.0
                        )
                        pb = ps_a.tile([128, COLS], F32, tag="pab", name="pab")
                        nc.tensor.matmul(
                            pb[:, 0:512], lsel,
                            uT[g][64 * ja : 64 * (ja + 1), 0:512],
                            start=True, stop=True,
                        )
                        nc.tensor.matmul(
                            pb[:, 512:768], lsel,
                            uT[g][64 * ja : 64 * (ja + 1), 512:768],
                            start=True, stop=True,
                        )
                        dBx = sp.tile([128, COLS], F32, tag="dBx", name="dBx")
                        nc.vector.tensor_tensor(
                            out=dBx[:, :].rearrange("p (b l) -> p b l", l=L),
                            in0=pb[:, :].rearrange("p (l b) -> p b l", b=QB),
                            in1=t_brep[:, :].rearrange("p (b l) -> p b l", l=L),
                            op=ALU.mult,
                        )
                        H = sp.tile([128, COLS], F32, tag="H", name="H")
                        nc.vector.tensor_tensor_scan(
                            H[:, :], dA[:, :], dBx[:, :], 0.0, ALU.mult, ALU.add
                        )
                        Hc = sp.tile([128, COLS], F32R, tag="Hc", name="Hc", bufs=2)
                        nc.gpsimd.tensor_tensor(
                            out=Hc[:, :], in0=H[:, :], in1=t_crep[:, :], op=ALU.mult
                        )
                        nc.tensor.matmul(
                            py[:, 0:512], t_selred[:, 128 * jj : 128 * (jj + 1)],
                            Hc[:, 0:512], start=(jj == 0), stop=(jj == 15),
                            skip_group_check=True,
                        )
                        nc.tensor.matmul(
                            py[:, 512:768], t_selred[:, 128 * jj : 128 * (jj + 1)],
                            Hc[:, 512:768], start=(jj == 0), stop=(jj == 15),
                            skip_group_check=True,
                        )
                    for c in range(2):
                        y1 = wp2.tile([128, HCH], F32, tag="y1", name="y1", bufs=1)
                        nc.vector.scalar_tensor_tensor(
                            out=y1[:, :].rearrange("p (b l) -> p b l", l=L),
                            in0=xcT[g][:, :].bitcast(F32).rearrange(
                                "p (l b) -> p b l", b=QB
                            )[:, 32 * c : 32 * (c + 1), :],
                            scalar=t_dp[:, g : g + 1],
                            in1=py[:, HCH * c : HCH * (c + 1)].rearrange(
                                "p (b l) -> p b l", l=L
                            ),
                            op0=ALU.mult, op1=ALU.add,
                        )
                        nc.vector.tensor_tensor(
                            out=y2T[g][:, HCH * c : HCH * (c + 1)].rearrange(
                                "p (b l) -> p b l", l=L
                            ),
                            in0=y1[:, :].rearrange("p (b l) -> p b l", l=L),
                            in1=szT[g][:, :].rearrange(
                                "p (l b) -> p b l", b=QB
                            )[:, 32 * c : 32 * (c + 1), :],
                            op=ALU.mult,
                        )

                # Phase G: out_proj (scaled by 1/L) + pooling
                for m in range(2):
                    for c in range(2):
                        p = ps_mm.tile([128, HCH], F32, tag="mm", name="mm")
                        for k in range(4):
                            nc.tensor.matmul(
                                p[:, :], t_outproj[k][:, 128 * m : 128 * (m + 1)],
                                y2T[k][:, HCH * c : HCH * (c + 1)],
                                start=(k == 0), stop=(k == 3),
                            )
                        with nc.allow_low_precision(reason="f32r out is fp32 bits"):
                            nc.vector.reduce_sum(
                                t_pooled[m][:, QB * q + 32 * c : QB * q + 32 * (c + 1)],
                                p[:, :].rearrange("p (b l) -> p b l", l=L),
                                axis=AX.X,
                            )

            qctx.close()

            # --- head ---
            with tc.tile_pool(name="headc", bufs=1) as hc, tc.tile_pool(
                name="headw", bufs=2
            ) as hw:
                t_gate = [
                    load(hc, d_gate[128 * k : 128 * (k + 1), :], (128, 1280), tag=f"g{k}")
                    for k in range(4)
                ]
                t_gbn = load(hc, d_gbn, (128, 10), F32, tag="gbn")
                t_avT = [
                    load(hc, d_avT[128 * k : 128 * (k + 1), :], (128, 1536), tag=f"av{k}")
                    for k in range(2)
                ]
                t_aoT = [
                    load(hc, d_aoT[128 * k : 128 * (k + 1), :], (128, 1536), tag=f"ao{k}")
                    for k in range(2)
                ]
                t_avb = load(hc, d_avb, (128, 12), F32, tag="avb")
                t_aob = load(hc, d_aob, (128, 12), F32, tag="aob")
                t_clsT = [
                    load(hc, d_clsT[128 * k : 128 * (k + 1), :], (128, 2415), tag=f"cl{k}")
                    for k in range(2)
                ]
                t_clsb = load(hc, d_clsb, (128, 22), F32, tag="clsb")

                prevT = None
                feats = []
                for i in range(6):
                    if i == 0:
                        srcT = t_pooled
                    else:
                        srcT = [hw.tile([128, BC], F32R, tag=f"src{m}", name=f"src{m}") for m in range(2)]
                        for m in range(2):
                            pg = ps_mm.tile([128, HCH], F32, tag="mm", name="mm")
                            for k in range(4):
                                rhs = prevT[k] if k < 2 else t_pooled[k - 2]
                                nc.tensor.matmul(
                                    pg[:, 0:BC],
                                    t_gate[k][
                                        :, DM * (i - 1) + 128 * m : DM * (i - 1) + 128 * (m + 1)
                                    ],
                                    rhs[:, :],
                                    start=(k == 0), stop=(k == 3),
                                )
                            e = hw.tile([128, BC], F32, tag="ge", name="ge")
                            nc.scalar.activation(
                                e[:, :], pg[:, 0:BC], AF.Exp, scale=-1.0,
                                bias=t_gbn[:, 2 * (i - 1) + m : 2 * (i - 1) + m + 1],
                            )
                            f1 = hw.tile([128, BC], F32, tag="gf", name="gf")
                            nc.vector.tensor_scalar(
                                out=f1[:, :], in0=e[:, :], scalar1=1.0, scalar2=None,
                                op0=ALU.add,
                            )
                            gsig = hw.tile([128, BC], F32, tag="gsig", name="gsig")
                            nc.vector.reciprocal(gsig[:, :], f1[:, :])
                            ddif = hw.tile([128, BC], F32, tag="gd", name="gd")
                            nc.vector.tensor_tensor(
                                out=ddif[:, :], in0=prevT[m][:, :].bitcast(F32),
                                in1=t_pooled[m][:, :].bitcast(F32), op=ALU.subtract,
                            )
                            s1 = hw.tile([128, BC], F32, tag="gs1", name="gs1")
                            nc.vector.tensor_tensor(
                                out=s1[:, :], in0=gsig[:, :], in1=ddif[:, :], op=ALU.mult
                            )
                            nc.vector.tensor_tensor(
                                out=srcT[m][:, :], in0=s1[:, :],
                                in1=t_pooled[m][:, :].bitcast(F32), op=ALU.add,
                            )
                    vT = [hw.tile([128, BC], F32R, tag=f"vT{m}", name=f"vT{m}") for m in range(2)]
                    for m in range(2):
                        pv = ps_mm.tile([128, HCH], F32, tag="mm", name="mm")
                        for k in range(2):
                            nc.tensor.matmul(
                                pv[:, 0:BC],
                                t_avT[k][:, DM * i + 128 * m : DM * i + 128 * (m + 1)],
                                srcT[k][:, :],
                                start=(k == 0), stop=(k == 1),
                            )
                        nc.scalar.activation(
                            vT[m][:, :], pv[:, 0:BC], AF.Identity,
                            bias=t_avb[:, 2 * i + m : 2 * i + m + 1],
                        )
                    newprev = [
                        hw.tile([128, BC], F32R, tag=f"ft{i}_{m}", name=f"ft{i}_{m}", bufs=1) for m in range(2)
                    ]
                    for m in range(2):
                        po = ps_mm.tile([128, HCH], F32, tag="mm", name="mm")
                        for k in range(2):
                            nc.tensor.matmul(
                                po[:, 0:BC],
                                t_aoT[k][:, DM * i + 128 * m : DM * i + 128 * (m + 1)],
                                vT[k][:, :],
                                start=(k == 0), stop=(k == 1),
                            )
                        nc.scalar.activation(
                            newprev[m][:, :], po[:, 0:BC], AF.Identity,
                            bias=t_aob[:, 2 * i + m : 2 * i + m + 1],
                        )
                    prevT = newprev
                    feats.append(newprev)

                # classifiers: logitsT = cls_w @ feat + b, then transpose to
                # row-major [b, ncls] and DMA out
                out_rows = [
                    [hw.tile([128, NCLS[i]], F32, tag=f"or{i}_{bb}", name=f"or{i}_{bb}", bufs=1) for bb in range(2)]
                    for i in range(6)
                ]
                for (lvl, moff, msz, bcol) in cls_chunks:
                    pc = ps_mm.tile([128, HCH], F32, tag="mm", name="mm")
                    for k in range(2):
                        nc.tensor.matmul(
                            pc[0:msz, 0:BC],
                            t_clsT[k][:, moff : moff + msz],
                            feats[lvl][k][:, :],
                            start=(k == 0), stop=(k == 1),
                        )
                    logT = hw.tile([128, BC], F32, tag="logT", name="logT")
                    nc.scalar.activation(
                        logT[0:msz, :], pc[0:msz, 0:BC], AF.Identity,
                        bias=t_clsb[:msz, bcol : bcol + 1],
                    )
                    lvl_off = moff - sum(NCLS[:lvl])
                    for bb in range(2):
                        pt = ps_tr.tile([128, 128], F32, tag="mm", name="mm")
                        nc.tensor.transpose(
                            pt[:, 0:msz],
                            logT[0:msz, 128 * bb : 128 * (bb + 1)],
                            t_ident[0:msz, 0:msz].bitcast(F32),
                        )
                        nc.scalar.copy(
                            out_rows[lvl][bb][:, lvl_off : lvl_off + msz],
                            pt[:, 0:msz],
                        )
                for i in range(6):
                    for bb in range(2):
                        nc.sync.dma_start(
                            d_out[i][128 * bb : 128 * (bb + 1), :],
                            out_rows[i][bb][:, :],
                        )
        # end ExitStack
    return nc, din


_CACHE = {}


def _get_program():
    if "prog" not in _CACHE:
        # cls_chunks layout is static
        col = 0
        off = 0
        cls_chunks = []
        for i, n in enumerate(NCLS):
            for mc in range(_cdiv(n, 128)):
                sz = min(128, n - 128 * mc)
                cls_chunks.append((i, off + 128 * mc, sz, col))
                col += 1
            off += n
        nc, din = build_program(cls_chunks)
        _CACHE["prog"] = nc
    return _CACHE["prog"]


def make_in_maps(inputs):
    """Split full inputs into per-core input maps (host-side prep)."""
    w = {k: np.asarray(v, dtype=np.float32) if not isinstance(v, tuple) else v
         for k, v in inputs.items()}
    w["cls_w"] = tuple(np.asarray(x, dtype=np.float32) for x in inputs["cls_w"])
    w["cls_b"] = tuple(np.asarray(x, dtype=np.float32) for x in inputs["cls_b"])
    consts, _ = _host_prep(w)
    coi = np.asarray(inputs["coi"], dtype=np.float32)
    mae = np.asarray(inputs["coi_mae"], dtype=np.float32)
    in_maps = []
    for c in range(NCORES):
        m = dict(consts)
        m["coi"] = np.ascontiguousarray(coi[BC * c : BC * (c + 1)])
        m["coi_mae"] = np.ascontiguousarray(mae[BC * c : BC * (c + 1)])
        in_maps.append(m)
    return in_maps


def kernel(**inputs):
    from concourse.bass_utils import run_bass_kernel_spmd

    nc = _get_program()
    in_maps = make_in_maps(inputs)
    res = run_bass_kernel_spmd(nc, in_maps, core_ids=list(range(NCORES)), trace=False)
    outs = []
    for i, n in enumerate(NCLS):
        full = np.concatenate(
            [res.results[c][f"out{i}"] for c in range(NCORES)], axis=0
        )
        outs.append(full)
    return tuple(outs)


# revision 19
# speedup vs baseline: 1.0346x; 1.0346x over previous
"""Trainium2 Bass kernel for the CurriculumDMGHANmae model.

Data-parallel over batch across 8 NeuronCores (256 samples/core). Per core,
the whole network runs in a feature-major layout (features on partitions,
batch*seq on the free dim) so every projection is a PE matmul with no
activations transposed except the initial input transpose.

The Mamba selective scan uses the DVE tensor_tensor_scan instruction: with
partitions = (d, n) state pairs and free = (sample, seqpos), one scan
instruction per 128-partition tile computes the whole recurrence; zeroing
the multiplier at every l==0 column makes the cross-sample scan exact.

Matmuls run in float32r (full-rate PE fp32, ~2^-13 input rounding); the
scan itself and all elementwise math stay in fp32.
"""
import sys

sys.path.insert(0, "/opt/trn_rl_repo")

import numpy as np
import orjson

import concourse.bass as bass
import concourse.bass2jax as bass2jax
import concourse.bass_utils as bass_utils
import concourse.mybir as mybir
import concourse.tile as tile
import bass_rust

F32 = mybir.dt.float32
F32R = mybir.dt.float32r
AF = mybir.ActivationFunctionType
ALU = mybir.AluOpType
AX = mybir.AxisListType

# ---------------------------------------------------------------------------
# Toolchain workarounds (this walrus build rejects >1 sync wait per
# instruction, and the Tile exit drain carries one wait per logical
# processor). Hoist excess waits onto same-engine NoOps at the BIR level.
# ---------------------------------------------------------------------------
_MAXW = 1
_ctr = [0]


def _split_waits_json(bir_bytes):
    d = orjson.loads(bir_bytes)
    changed = False
    for fn in d.get("functions", []):
        for blk in fn.get("blocks", []):
            out = []
            for ins in blk.get("instructions", []):
                si = ins.get("sync_info")
                waits = si.get("on_wait") if si else None
                if waits and len(waits) > _MAXW:
                    extra = waits[:-_MAXW]
                    si["on_wait"] = waits[-_MAXW:]
                    for i in range(0, len(extra), _MAXW):
                        _ctr[0] += 1
                        out.append(
                            {
                                "engine": ins["engine"],
                                "ins": [],
                                "name": f"I-waitsplit-{_ctr[0]}",
                                "opcode": "NoOp",
                                "outs": [],
                                "sync_info": {
                                    "on_update": [],
                                    "on_wait": extra[i : i + _MAXW],
                                },
                            }
                        )
                    changed = True
                out.append(ins)
            blk["instructions"] = out
    return orjson.dumps(d) if changed else bir_bytes


_orig_compile_bir_kernel = bass_utils.compile_bir_kernel


def _patched_compile_bir_kernel(bir_json, tmpdir, neff_name="file.neff"):
    if isinstance(bir_json, str):
        bir_json = bir_json.encode()
    return _orig_compile_bir_kernel(
        _split_waits_json(bir_json), tmpdir, neff_name=neff_name
    )


def _patched_drain_and_barrier(self, tick_clock, wait_clock):
    nc = self.nc
    probe = nc.sync.nop()
    wait_clock.add_sem_waits(
        probe.ins, tile.ScopedClock({None: tick_clock.global_clock})
    )
    si = probe.ins.sync_info
    waits = list(si.on_wait) if si is not None else []
    if len(waits) > 1:
        probe.ins.sync_info = bass_rust.SyncInfo(on_wait=waits[:1], on_update=[])
        for w in waits[1:]:
            extra = nc.sync.nop()
            extra.ins.sync_info = bass_rust.SyncInfo(on_wait=[w], on_update=[])
    nc.sync.drain()
    nc.all_engine_barrier()
    assert self.sems is not None
    popped = nc._tile_sem_poison_stack.pop()
    assert popped is self._sem_poison
    nc.clear_and_free_semaphores(list(self.sems.allocated().values()))
    nc.all_engine_barrier()


def _apply_patches():
    bass_utils.compile_bir_kernel = _patched_compile_bir_kernel
    bass2jax.compile_bir_kernel = _patched_compile_bir_kernel
    tile.TileContext._drain_and_barrier = _patched_drain_and_barrier


_apply_patches()

# ---------------------------------------------------------------------------
# Model constants
# ---------------------------------------------------------------------------
NCORES = 8
B_FULL = 2048
BC = B_FULL // NCORES  # 256 samples per core
L = 12
DM = 256
DI = 512
DS = 16
DTR = 16
NCLS = (5, 30, 80, 200, 600, 1500)
LN_EPS = 1e-5
NQ = 4  # quarters per core
QB = BC // NQ  # 64 samples per quarter
COLS = QB * L  # 768 free columns per quarter
HCH = 384  # psum chunk (half of COLS)


def _cdiv(a, b):
    return (a + b - 1) // b


def _host_prep(w):
    """Build all device-side constant arrays from the raw model weights."""
    f32 = lambda x: np.ascontiguousarray(x, dtype=np.float32)
    WfL = w["fusion_w"][:, :DM]
    WfR = w["fusion_w"][:, DM:]
    out = {}
    out["w1f_T"] = f32((WfL @ w["coi_proj_w"]).T)  # [768, 256]
    out["w2f_T"] = f32((WfR @ w["coi_mae_proj_w"]).T)  # [768, 256]
    bias_fused = WfL @ w["coi_proj_b"] + WfR @ w["coi_mae_proj_b"] + w["fusion_b"]
    out["bias_fused"] = f32(bias_fused.reshape(2, 128).T)  # [128, 2]
    out["ln_g"] = f32(w["ln_g"].reshape(2, 128).T)
    out["ln_b"] = f32(w["ln_b"].reshape(2, 128).T)
    # Wbig: conv folded into the xc half of in_proj.
    Win_x = w["in_proj_w"][:DI]  # [512, 256]
    Win_z = w["in_proj_w"][DI:]  # [512, 256]
    wbig = np.zeros((4 * DM, DI), dtype=np.float32)
    for k in range(4):
        wbig[k * DM : (k + 1) * DM, :] = (w["conv_w"][:, k][:, None] * Win_x).T
    out["wbig_T"] = f32(wbig)  # [1024, 512]
    out["wz_T"] = f32(Win_z.T)  # [256, 512]
    out["conv_b_neg"] = f32(-w["conv_b"].reshape(4, 128).T)  # [128, 4]
    out["conv_b"] = f32(w["conv_b"].reshape(4, 128).T)
    out["x_proj_T"] = f32(w["x_proj_w"].T)  # [512, 48]
    out["dt_proj_T"] = f32(w["dt_proj_w"].T)  # [16, 512]
    out["dt_proj_b"] = f32(w["dt_proj_b"].reshape(4, 128).T)  # [128, 4]
    A = -np.exp(w["A_log"])  # [512, 16]
    acols = np.zeros((128, 64), dtype=np.float32)
    for j in range(64):
        for p in range(128):
            acols[p, j] = A[8 * j + p // 16, p % 16]
    out["a_cols"] = f32(acols)
    out["dp_col"] = f32(w["Dp"].reshape(4, 128).T)  # [128, 4]
    out["out_proj_T"] = f32(w["out_proj_w"].T / float(L))  # [512, 256], pool fold
    selrep = np.zeros((128, 1024), dtype=np.float32)
    for a in range(2):
        for m in range(8):
            for p in range(128):
                selrep[64 * a + 8 * m + p // 16, 128 * m + p] = 1.0
    out["selrep"] = selrep  # [128,1024]: same [64,128] pattern at bases 0/64
    sel16b = np.zeros((48, 128), dtype=np.float32)
    for p in range(128):
        sel16b[16 + p % 16, p] = 1.0
    out["sel16b"] = sel16b  # vs dbcT[0:48]: picks rows 16..31 (B)
    sel16c = np.zeros((48, 128), dtype=np.float32)
    for p in range(128):
        sel16c[32 + p % 16, p] = 1.0
    out["sel16c"] = sel16c  # vs dbcT[0:48]: picks rows 32..47 (C)
    selred = np.zeros((128, 16 * 128), dtype=np.float32)
    for jj in range(16):
        for p in range(128):
            selred[p, 128 * jj + 8 * jj + p // 16] = 1.0
    out["selred"] = f32(selred)  # [128, 2048], slice jj = [:, 128jj:128jj+128]
    out["ones_red"] = f32(np.full((128, 1), 1.0 / DM, dtype=np.float32))
    out["ones_row"] = f32(np.ones((1, 128), dtype=np.float32))
    out["eps_col"] = f32(np.full((128, 1), LN_EPS, dtype=np.float32))
    out["ident"] = f32(np.eye(128, dtype=np.float32))
    # head
    gate_T = np.zeros((DI, 5 * DM), dtype=np.float32)
    for i in range(5):
        gate_T[:, DM * i : DM * (i + 1)] = w["gate_w"][i].T
    out["gate_T"] = f32(gate_T)
    out["gate_b_neg"] = f32(
        -np.stack([w["gate_b"][i].reshape(2, 128).T for i in range(5)], 0)
        .transpose(1, 0, 2)
        .reshape(128, 10)
    )  # col 2i+m
    attn_v_T = np.zeros((DM, 6 * DM), dtype=np.float32)
    attn_o_T = np.zeros((DM, 6 * DM), dtype=np.float32)
    attn_vb = np.zeros((128, 12), dtype=np.float32)
    attn_ob = np.zeros((128, 12), dtype=np.float32)
    for i in range(6):
        wv = w["attn_in_w"][i][2 * DM :]  # [256, 256]
        bv = w["attn_in_b"][i][2 * DM :]
        attn_v_T[:, DM * i : DM * (i + 1)] = wv.T
        attn_o_T[:, DM * i : DM * (i + 1)] = w["attn_out_w"][i].T
        attn_vb[:, 2 * i : 2 * i + 2] = bv.reshape(2, 128).T
        attn_ob[:, 2 * i : 2 * i + 2] = w["attn_out_b"][i].reshape(2, 128).T
    out["attn_v_T"] = f32(attn_v_T)
    out["attn_o_T"] = f32(attn_o_T)
    out["attn_vb"] = f32(attn_vb)
    out["attn_ob"] = f32(attn_ob)
    cls_T = np.zeros((DM, sum(NCLS)), dtype=np.float32)
    off = 0
    cls_chunks = []  # (level, m_off_global, m_size, bias_col)
    cls_b = np.zeros((128, 22), dtype=np.float32)
    col = 0
    for i, n in enumerate(NCLS):
        cls_T[:, off : off + n] = w["cls_w"][i].T
        for mc in range(_cdiv(n, 128)):
            sz = min(128, n - 128 * mc)
            cls_b[:sz, col] = w["cls_b"][i][128 * mc : 128 * mc + sz]
            cls_chunks.append((i, off + 128 * mc, sz, col))
            col += 1
        off += n
    out["cls_T"] = f32(cls_T)
    out["cls_b"] = f32(cls_b)
    return out, cls_chunks


# ---------------------------------------------------------------------------
# Program builder
# ---------------------------------------------------------------------------
def build_program(cls_chunks):
    nc = bass.Bass("TRN2", debug=False, num_devices=NCORES)

    din = {}

    def dram_in(name, shape, dt=F32R):
        din[name] = nc.dram_tensor(name, list(shape), dt, kind="ExternalInput").ap()
        return din[name]

    d_coi = dram_in("coi", (BC, L, 768)).rearrange("b l f -> (b l) f")
    d_mae = dram_in("coi_mae", (BC, 768))
    d_w1f = dram_in("w1f_T", (768, 256))
    d_w2f = dram_in("w2f_T", (768, 256))
    d_biasf = dram_in("bias_fused", (128, 2), F32)
    d_lng = dram_in("ln_g", (128, 2), F32)
    d_lnb = dram_in("ln_b", (128, 2), F32)
    d_wbig = dram_in("wbig_T", (1024, 512))
    d_wz = dram_in("wz_T", (256, 512))
    d_convbn = dram_in("conv_b_neg", (128, 4), F32)
    d_convb = dram_in("conv_b", (128, 4), F32)
    d_xproj = dram_in("x_proj_T", (512, 48))
    d_dtproj = dram_in("dt_proj_T", (16, 512))
    d_dtb = dram_in("dt_proj_b", (128, 4), F32)
    d_acols = dram_in("a_cols", (128, 64), F32)
    d_dp = dram_in("dp_col", (128, 4), F32)
    d_outproj = dram_in("out_proj_T", (512, 256))
    d_selrep = dram_in("selrep", (128, 1024))
    d_sel16b = dram_in("sel16b", (48, 128))
    d_sel16c = dram_in("sel16c", (48, 128))
    d_selred = dram_in("selred", (128, 2048))
    d_onesred = dram_in("ones_red", (128, 1))
    d_onesrow = dram_in("ones_row", (1, 128))
    d_epscol = dram_in("eps_col", (128, 1), F32)
    d_ident = dram_in("ident", (128, 128))
    d_gate = dram_in("gate_T", (512, 1280))
    d_gbn = dram_in("gate_b_neg", (128, 10), F32)
    d_avT = dram_in("attn_v_T", (256, 1536))
    d_aoT = dram_in("attn_o_T", (256, 1536))
    d_avb = dram_in("attn_vb", (128, 12), F32)
    d_aob = dram_in("attn_ob", (128, 12), F32)
    d_clsT = dram_in("cls_T", (256, 2415))
    d_clsb = dram_in("cls_b", (128, 22), F32)

    d_out = [
        nc.dram_tensor(f"out{i}", [BC, n], F32, kind="ExternalOutput").ap()
        for i, n in enumerate(NCLS)
    ]

    with tile.TileContext(nc) as tc:
        from contextlib import ExitStack

        with ExitStack() as ctx:
            cp = ctx.enter_context(tc.tile_pool(name="consts", bufs=1))
            pp = ctx.enter_context(tc.tile_pool(name="persist", bufs=1))

            def load(pool, d, shape, dt=F32R, tag=None):
                t_name = tag or f"c{len(din)}_{id(d)}"
                t = pool.tile(list(shape), dt, tag=t_name, name=t_name)
                nc.sync.dma_start(t[:, :], d)
                return t

            # --- constants ---
            t_w1f = [
                load(cp, d_w1f[128 * k : 128 * (k + 1), :], (128, 256), tag=f"w1f{k}")
                for k in range(6)
            ]
            t_w2f = [
                load(cp, d_w2f[128 * k : 128 * (k + 1), :], (128, 256), tag=f"w2f{k}")
                for k in range(6)
            ]
            t_biasf = load(cp, d_biasf, (128, 2), F32, tag="biasf")
            t_lng = load(cp, d_lng, (128, 2), F32, tag="lng")
            t_lnb = load(cp, d_lnb, (128, 2), F32, tag="lnb")
            t_wbig = [
                load(cp, d_wbig[128 * k : 128 * (k + 1), :], (128, 512), tag=f"wbig{k}")
                for k in range(8)
            ]
            t_wz = [
                load(cp, d_wz[128 * k : 128 * (k + 1), :], (128, 512), tag=f"wz{k}")
                for k in range(2)
            ]
            t_convbn = load(cp, d_convbn, (128, 4), F32, tag="convbn")
            t_convb = load(cp, d_convb, (128, 4), F32, tag="convb")
            t_xproj = [
                load(cp, d_xproj[128 * k : 128 * (k + 1), :], (128, 48), tag=f"xp{k}")
                for k in range(4)
            ]
            t_dtproj = load(cp, d_dtproj, (16, 512), tag="dtp")
            t_dtb = load(cp, d_dtb, (128, 4), F32, tag="dtb")
            t_acols = load(cp, d_acols, (128, 64), F32, tag="acols")
            t_dp = load(cp, d_dp, (128, 4), F32, tag="dp")
            t_outproj = [
                load(cp, d_outproj[128 * k : 128 * (k + 1), :], (128, 256), tag=f"op{k}")
                for k in range(4)
            ]
            t_selrep = load(cp, d_selrep, (128, 1024), tag="selrep")
            t_sel16b = load(cp, d_sel16b, (48, 128), tag="sel16b")
            t_sel16c = load(cp, d_sel16c, (48, 128), tag="sel16c")
            t_selred = load(cp, d_selred, (128, 2048), tag="selred")
            t_onesred = load(cp, d_onesred, (128, 1), tag="onesred")
            t_onesrow = load(cp, d_onesrow, (1, 128), tag="onesrow")
            t_epscol = load(cp, d_epscol, (128, 1), F32, tag="epscol")
            t_ident = load(cp, d_ident, (128, 128), tag="ident")

            # persistent across quarters
            t_pooled = [pp.tile([128, BC], F32R, tag=f"pooled{m}", name=f"pooled{m}") for m in range(2)]

            ps_mm = ctx.enter_context(
                tc.tile_pool(name="ps_mm", bufs=2, space="PSUM")
            )
            ps_tr = ps_mm
            ps_strip = ps_mm

            # --- stage 0: mae path ---
            with tc.tile_pool(name="mae", bufs=1) as mp:
                rows = [
                    load(mp, d_mae[128 * r : 128 * (r + 1), :], (128, 768), tag=f"mr{r}")
                    for r in range(2)
                ]
                maeT = [mp.tile([128, 256], F32R, tag=f"maeT{f}", name=f"maeT{f}") for f in range(6)]
                for f in range(6):
                    p = ps_tr.tile([128, 256], F32R, tag="mm", name="mm")
                    for r in range(2):
                        nc.tensor.transpose(
                            p[:, 128 * r : 128 * (r + 1)],
                            rows[r][:, 128 * f : 128 * (f + 1)],
                            t_ident[:, :],
                        )
                    nc.scalar.copy(maeT[f][:, :], p[:, :].bitcast(F32))
                t_maef = [
                    pp.tile([128, BC], F32, tag=f"maef{m}", name=f"maef{m}") for m in range(2)
                ]
                for m in range(2):
                    p = ps_mm.tile([128, 384], F32, tag="mm", name="mm")
                    for k in range(6):
                        nc.tensor.matmul(
                            p[:, 0:256],
                            t_w2f[k][:, 128 * m : 128 * (m + 1)],
                            maeT[k][:, :],
                            start=(k == 0),
                            stop=(k == 5),
                        )
                    nc.scalar.activation(
                        t_maef[m][:, :], p[:, 0:256], AF.Identity,
                        bias=t_biasf[:, m : m + 1],
                    )

            # --- quarters ---
            qctx = ExitStack()
            wp = qctx.enter_context(tc.tile_pool(name="work", bufs=1))
            wp2 = qctx.enter_context(tc.tile_pool(name="work2", bufs=2))
            sp = qctx.enter_context(tc.tile_pool(name="scan", bufs=2))
            ps_a = qctx.enter_context(tc.tile_pool(name="ps_a", bufs=2, space="PSUM"))
            ps_y = qctx.enter_context(tc.tile_pool(name="ps_y", bufs=1, space="PSUM"))

            for q in range(NQ):
                # Phase A: load + transpose coi
                coiT = [wp2.tile([128, COLS], F32R, tag=f"coiT{f}", name=f"coiT{f}", bufs=1) for f in range(6)]
                for r in range(6):
                    rt = wp2.tile([128, 768], F32R, tag="coirow", name="coirow", bufs=1)
                    # rows of this tile are (l, b) l-major: 2 l values x 64 b
                    for i in range(2):
                        nc.sync.dma_start(
                            rt[64 * i : 64 * (i + 1), :],
                            d_coi.rearrange("(b l) f -> b l f", l=L)[
                                QB * q : QB * (q + 1), 2 * r + i, :
                            ],
                        )
                    for fp in range(3):
                        p = ps_tr.tile([128, 256], F32R, tag="mm", name="mm")
                        for f2 in range(2):
                            f = 2 * fp + f2
                            nc.tensor.transpose(
                                p[:, 128 * f2 : 128 * (f2 + 1)],
                                rt[:, 128 * f : 128 * (f + 1)],
                                t_ident[:, :],
                            )
                        for f2 in range(2):
                            f = 2 * fp + f2
                            nc.scalar.copy(
                                coiT[f][:, 128 * r : 128 * (r + 1)],
                                p[:, 128 * f2 : 128 * (f2 + 1)].bitcast(F32),
                            )

                # Phase B: fused projection + mae + bias + relu
                fusedT = [wp.tile([128, COLS], F32R, tag=f"fusedT{m}", name=f"fusedT{m}") for m in range(2)]
                for m in range(2):
                    for c in range(2):
                        p = ps_mm.tile([128, HCH], F32, tag="mm", name="mm")
                        for k in range(6):
                            nc.tensor.matmul(
                                p[:, :],
                                t_w1f[k][:, 128 * m : 128 * (m + 1)],
                                coiT[k][:, HCH * c : HCH * (c + 1)],
                                start=(k == 0),
                                stop=(k == 5),
                            )
                        tmp = wp2.tile([128, HCH], F32, tag="y1", name="btmp", bufs=1)
                        nc.vector.scalar_tensor_tensor(
                            out=tmp[:, :].rearrange("p (l b) -> p b l", b=QB),
                            in0=p[:, :].rearrange("p (l b) -> p b l", b=QB),
                            scalar=t_biasf[:, m : m + 1],
                            in1=t_maef[m][:, QB * q : QB * (q + 1)]
                            .broadcast_to([128, QB, 6]),
                            op0=ALU.add,
                            op1=ALU.add,
                        )
                        nc.scalar.activation(
                            fusedT[m][:, HCH * c : HCH * (c + 1)], tmp[:, :], AF.Relu
                        )

                # Phase C: LayerNorm
                lnT = [wp2.tile([128, COLS], F32R, tag=f"coiT{m}", name=f"lnT{m}", bufs=1) for m in range(2)]
                sqT = [wp2.tile([128, COLS], F32R, tag=f"sqT{m}", name=f"sqT{m}", bufs=1) for m in range(2)]
                for m in range(2):
                    nc.scalar.activation(sqT[m][:, :], fusedT[m][:, :].bitcast(F32), AF.Square)
                t2T = [wp2.tile([128, COLS], F32, tag=f"t2T{m}", name=f"t2T{m}", bufs=1) for m in range(2)]
                reps = []
                for c in range(2):
                    pmean = ps_strip.tile([1, HCH], F32, tag="mm", name="pmean")
                    pex2 = ps_strip.tile([1, HCH], F32, tag="mm", name="pex2")
                    for m in range(2):
                        nc.tensor.matmul(
                            pmean[:, :], t_onesred[:, :],
                            fusedT[m][:, HCH * c : HCH * (c + 1)],
                            start=(m == 0), stop=(m == 1),
                        )
                        nc.tensor.matmul(
                            pex2[:, :], t_onesred[:, :],
                            sqT[m][:, HCH * c : HCH * (c + 1)],
                            start=(m == 0), stop=(m == 1),
                        )
                    mean_sb = wp2.tile([1, HCH], F32R, tag="mean_sb", name="mean_sb", bufs=2)
                    nc.scalar.copy(mean_sb[:, :], pmean[:, :])
                    msq = wp2.tile([1, HCH], F32, tag="msq", name="msq")
                    nc.vector.tensor_tensor(
                        out=msq[:, :], in0=mean_sb[:, :].bitcast(F32),
                        in1=mean_sb[:, :].bitcast(F32), op=ALU.mult,
                    )
                    var = wp2.tile([1, HCH], F32, tag="var", name="var")
                    nc.vector.tensor_tensor(
                        out=var[:, :], in0=pex2[:, :], in1=msq[:, :], op=ALU.subtract,
                    )
                    lnv = wp2.tile([1, HCH], F32, tag="lnv", name="lnv")
                    nc.scalar.activation(
                        lnv[:, :], var[:, :], AF.Ln, bias=t_epscol[0:1, :]
                    )
                    rstd = wp2.tile([1, HCH], F32R, tag="rstd", name="rstd", bufs=2)
                    nc.scalar.activation(rstd[:, :], lnv[:, :], AF.Exp, scale=-0.5)
                    reps.append((mean_sb, rstd))
                for c in range(2):
                    mean_sb, rstd = reps[c]
                    prep_m = ps_strip.tile([128, HCH], F32, tag="mm", name="prepm")
                    nc.tensor.matmul(
                        prep_m[:, :], t_onesrow[:, :], mean_sb[:, :],
                        start=True, stop=True,
                    )
                    prep_r = ps_strip.tile([128, HCH], F32, tag="mm", name="prepr")
                    nc.tensor.matmul(
                        prep_r[:, :], t_onesrow[:, :], rstd[:, :],
                        start=True, stop=True,
                    )
                    for m in range(2):
                        t1 = wp2.tile([128, HCH], F32, tag="lnt1", name="lnt1", bufs=1)
                        nc.vector.tensor_tensor(
                            out=t1[:, :],
                            in0=fusedT[m][:, HCH * c : HCH * (c + 1)].bitcast(F32),
                            in1=prep_m[:, :], op=ALU.subtract,
                        )
                        nc.vector.tensor_tensor(
                            out=t2T[m][:, HCH * c : HCH * (c + 1)],
                            in0=t1[:, :], in1=prep_r[:, :], op=ALU.mult,
                        )
                for m in range(2):
                    nc.scalar.activation(
                        lnT[m][:, :], t2T[m][:, :], AF.Identity,
                        bias=t_lnb[:, m : m + 1], scale=t_lng[:, m : m + 1],
                    )

                # Phase D: in_proj with folded conv (xc) and z + silu
                xcT = [wp.tile([128, COLS], F32R, tag=f"xcT{g}", name=f"xcT{g}") for g in range(4)]
                szT = [wp.tile([128, COLS], F32, tag=f"szT{g}", name=f"szT{g}") for g in range(4)]
                for g in range(4):
                    for c in range(2):
                        p = ps_mm.tile([128, HCH], F32, tag="mm", name="mm")
                        # shift k=3 (offset 0), full range, starts accumulation
                        for h in range(2):
                            nc.tensor.matmul(
                                p[:, :],
                                t_wbig[6 + h][:, 128 * g : 128 * (g + 1)],
                                lnT[h][:, HCH * c : HCH * (c + 1)],
                                start=(h == 0), stop=False,
                                skip_group_check=True,
                            )
                        for k in (2, 1, 0):
                            o = 3 - k
                            for h in range(2):
                                if c == 0:
                                    outap = p[:, QB * o : HCH]
                                    rhsap = lnT[h][:, 0 : HCH - QB * o]
                                else:
                                    outap = p[:, :]
                                    rhsap = lnT[h][:, HCH - QB * o : 2 * HCH - QB * o]
                                nc.tensor.matmul(
                                    outap,
                                    t_wbig[2 * k + h][:, 128 * g : 128 * (g + 1)],
                                    rhsap,
                                    start=False, stop=(k == 0 and h == 1),
                                    skip_group_check=True,
                                )
                        # silu evac: xc = (p + b) * sigmoid(p + b)
                        e = wp2.tile([128, HCH], F32, tag="se", name="se", bufs=1)
                        nc.scalar.activation(
                            e[:, :], p[:, :], AF.Exp, scale=-1.0,
                            bias=t_convbn[:, g : g + 1],
                        )
                        f1 = wp2.tile([128, HCH], F32, tag="sf", name="sf", bufs=1)
                        nc.vector.tensor_scalar(
                            out=f1[:, :], in0=e[:, :], scalar1=1.0, scalar2=None,
                            op0=ALU.add,
                        )
                        r1 = wp2.tile([128, HCH], F32, tag="sr", name="sr", bufs=1)
                        nc.vector.reciprocal(r1[:, :], f1[:, :])
                        nc.vector.scalar_tensor_tensor(
                            out=xcT[g][:, HCH * c : HCH * (c + 1)],
                            in0=p[:, :], scalar=t_convb[:, g : g + 1],
                            in1=r1[:, :], op0=ALU.add, op1=ALU.mult,
                        )
                for g in range(4):
                    for c in range(2):
                        p = ps_mm.tile([128, HCH], F32, tag="mm", name="mm")
                        for h in range(2):
                            nc.tensor.matmul(
                                p[:, :],
                                t_wz[h][:, 128 * g : 128 * (g + 1)],
                                lnT[h][:, HCH * c : HCH * (c + 1)],
                                start=(h == 0), stop=(h == 1),
                            )
                        e = wp2.tile([128, HCH], F32, tag="se", name="se", bufs=1)
                        nc.scalar.activation(e[:, :], p[:, :], AF.Exp, scale=-1.0)
                        f1 = wp2.tile([128, HCH], F32, tag="sf", name="sf", bufs=1)
                        nc.vector.tensor_scalar(
                            out=f1[:, :], in0=e[:, :], scalar1=1.0, scalar2=None,
                            op0=ALU.add,
                        )
                        r1 = wp2.tile([128, HCH], F32, tag="sr", name="sr", bufs=1)
                        nc.vector.reciprocal(r1[:, :], f1[:, :])
                        nc.vector.scalar_tensor_tensor(
                            out=szT[g][:, HCH * c : HCH * (c + 1)],
                            in0=p[:, :], scalar=0.0,
                            in1=r1[:, :], op0=ALU.bypass, op1=ALU.mult,
                        )

                # Phase E: x_proj -> dbc; dt; u; B/C replication
                dbcT = wp.tile([48, COLS], F32R, tag="dbcT", name="dbcT")
                for c in range(2):
                    p = ps_mm.tile([128, HCH], F32, tag="mm", name="mm")
                    for k in range(4):
                        nc.tensor.matmul(
                            p[0:48, :], t_xproj[k][:, :],
                            xcT[k][:, HCH * c : HCH * (c + 1)],
                            start=(k == 0), stop=(k == 3),
                        )
                    nc.scalar.copy(
                        dbcT[:, HCH * c : HCH * (c + 1)], p[0:48, :]
                    )
                dtT = [wp.tile([128, COLS], F32R, tag=f"dtT{g}", name=f"dtT{g}") for g in range(4)]
                for g in range(4):
                    for c in range(2):
                        p = ps_mm.tile([128, HCH], F32, tag="mm", name="mm")
                        nc.tensor.matmul(
                            p[:, :], t_dtproj[:, 128 * g : 128 * (g + 1)],
                            dbcT[0:16, HCH * c : HCH * (c + 1)],
                            start=True, stop=True,
                        )
                        e = wp2.tile([128, HCH], F32, tag="spe", name="spe", bufs=1)
                        nc.scalar.activation(
                            e[:, :], p[:, :], AF.Exp, bias=t_dtb[:, g : g + 1]
                        )
                        nc.scalar.activation(
                            dtT[g][:, HCH * c : HCH * (c + 1)], e[:, :], AF.Ln,
                            bias=1.0,
                        )
                uT = [wp.tile([128, COLS], F32R, tag=f"uT{g}", name=f"uT{g}") for g in range(4)]
                for g in range(4):
                    nc.vector.tensor_tensor(
                        out=uT[g][:, :], in0=dtT[g][:, :].bitcast(F32),
                        in1=xcT[g][:, :].bitcast(F32), op=ALU.mult,
                    )
                t_brep = wp.tile([128, COLS], F32, tag="brep", name="brep")
                t_crep = wp.tile([128, COLS], F32, tag="crep", name="crep")
                for c in range(2):
                    pb = ps_mm.tile([128, HCH], F32, tag="mm", name="mm")
                    nc.tensor.matmul(
                        pb[:, :], t_sel16b[:, :], dbcT[0:48, HCH * c : HCH * (c + 1)],
                        start=True, stop=True,
                    )
                    nc.scalar.activation(
                        t_brep[:, :].rearrange("p (b l) -> p b l", l=L)[
                            :, :, 6 * c : 6 * (c + 1)
                        ],
                        pb[:, :].rearrange("p (l b) -> p b l", b=QB),
                        AF.Copy,
                    )
                    pc = ps_mm.tile([128, HCH], F32, tag="mm", name="mm")
                    nc.tensor.matmul(
                        pc[:, :], t_sel16c[:, :], dbcT[0:48, HCH * c : HCH * (c + 1)],
                        start=True, stop=True,
                    )
                    nc.scalar.activation(
                        t_crep[:, :].rearrange("p (b l) -> p b l", l=L)[
                            :, :, 6 * c : 6 * (c + 1)
                        ],
                        pc[:, :].rearrange("p (l b) -> p b l", b=QB),
                        AF.Copy,
                    )

                # Phase F: scan core over 64 dn-tiles
                y2T = [wp.tile([128, COLS], F32R, tag=(f"fusedT{g}" if g < 2 else f"y2T{g}"), name=f"y2T{g}") for g in range(4)]
                for g in range(4):
                    py = ps_y.tile([128, COLS], F32, tag="py", name="py")
                    for jj in range(16):
                        j = 16 * g + jj
                        prow = 8 * jj
                        ja, jm = jj // 8, jj % 8
                        lsel = t_selrep[64 * ja : 64 * (ja + 1), 128 * jm : 128 * (jm + 1)]
                        pa = ps_a.tile([128, COLS], F32, tag="pab", name="pab")
                        nc.tensor.matmul(
                            pa[:, 0:512], lsel,
                            dtT[g][64 * ja : 64 * (ja + 1), 0:512],
                            start=True, stop=True,
                        )
                        nc.tensor.matmul(
                            pa[:, 512:768], lsel,
                            dtT[g][64 * ja : 64 * (ja + 1), 512:768],
                            start=True, stop=True,
                        )
                        dA = sp.tile([128, COLS], F32, tag="dA", name="dA")
                        nc.scalar.activation(
                            dA[:, :].rearrange("p (b l) -> p b l", l=L),
                            pa[:, :].rearrange("p (l b) -> p b l", b=QB),
                            AF.Exp, scale=t_acols[:, j : j + 1],
                        )
                        nc.gpsimd.memset(
                            dA[:, :].rearrange("p (b l) -> p b l", l=L)[:, :, 0:1], 0.0
                        )
                        pb = ps_a.tile([128, COLS], F32, tag="pab", name="pab")
                        nc.tensor.matmul(
                            pb[:, 0:512], lsel,
                            uT[g][64 * ja : 64 * (ja + 1), 0:512],
                            start=True, stop=True,
                        )
                        nc.tensor.matmul(
                            pb[:, 512:768], lsel,
                            uT[g][64 * ja : 64 * (ja + 1), 512:768],
                            start=True, stop=True,
                        )
                        dBx = sp.tile([128, COLS], F32, tag="dBx", name="dBx")
                        nc.vector.tensor_tensor(
                            out=dBx[:, :].rearrange("p (b l) -> p b l", l=L),
                            in0=pb[:, :].rearrange("p (l b) -> p b l", b=QB),
                            in1=t_brep[:, :].rearrange("p (b l) -> p b l", l=L),
                            op=ALU.mult,
                        )
                        H = sp.tile([128, COLS], F32, tag="H", name="H")
                        nc.vector.tensor_tensor_scan(
                            H[:, :], dA[:, :], dBx[:, :], 0.0, ALU.mult, ALU.add
                        )
                        Hc = sp.tile([128, COLS], F32R, tag="Hc", name="Hc", bufs=2)
                        nc.gpsimd.tensor_tensor(
                            out=Hc[:, :], in0=H[:, :], in1=t_crep[:, :], op=ALU.mult
                        )
                        nc.tensor.matmul(
                            py[:, 0:512], t_selred[:, 128 * jj : 128 * (jj + 1)],
                            Hc[:, 0:512], start=(jj == 0), stop=(jj == 15),
                            skip_group_check=True,
                        )
                        nc.tensor.matmul(
                            py[:, 512:768], t_selred[:, 128 * jj : 128 * (jj + 1)],
                            Hc[:, 512:768], start=(jj == 0), stop=(jj == 15),
                            skip_group_check=True,
                        )
                    for c in range(2):
                        y1 = wp2.tile([128, HCH], F32, tag="y1", name="y1", bufs=1)
                        nc.vector.scalar_tensor_tensor(
                            out=y1[:, :].rearrange("p (b l) -> p b l", l=L),
                            in0=xcT[g][:, :].bitcast(F32).rearrange(
                                "p (l b) -> p b l", b=QB
                            )[:, 32 * c : 32 * (c + 1), :],
                            scalar=t_dp[:, g : g + 1],
                            in1=py[:, HCH * c : HCH * (c + 1)].rearrange(
                                "p (b l) -> p b l", l=L
                            ),
                            op0=ALU.mult, op1=ALU.add,
                        )
                        nc.vector.tensor_tensor(
                            out=y2T[g][:, HCH * c : HCH * (c + 1)].rearrange(
                                "p (b l) -> p b l", l=L
                            ),
                            in0=y1[:, :].rearrange("p (b l) -> p b l", l=L),
                            in1=szT[g][:, :].rearrange(
                                "p (l b) -> p b l", b=QB
                            )[:, 32 * c : 32 * (c + 1), :],
                            op=ALU.mult,
                        )

                # Phase G: out_proj (scaled by 1/L) + pooling
                for m in range(2):
                    for c in range(2):
                        p = ps_mm.tile([128, HCH], F32, tag="mm", name="mm")
                        for k in range(4):
                            nc.tensor.matmul(
                                p[:, :], t_outproj[k][:, 128 * m : 128 * (m + 1)],
                                y2T[k][:, HCH * c : HCH * (c + 1)],
                                start=(k == 0), stop=(k == 3),
                            )
                        with nc.allow_low_precision(reason="f32r out is fp32 bits"):
                            nc.vector.reduce_sum(
                                t_pooled[m][:, QB * q + 32 * c : QB * q + 32 * (c + 1)],
                                p[:, :].rearrange("p (b l) -> p b l", l=L),
                                axis=AX.X,
                            )

            qctx.close()

            # --- head ---
            with tc.tile_pool(name="headc", bufs=1) as hc, tc.tile_pool(
                name="headw", bufs=2
            ) as hw:
                t_gate = [
                    load(hc, d_gate[128 * k : 128 * (k + 1), :], (128, 1280), tag=f"g{k}")
                    for k in range(4)
                ]
                t_gbn = load(hc, d_gbn, (128, 10), F32, tag="gbn")
                t_avT = [
                    load(hc, d_avT[128 * k : 128 * (k + 1), :], (128, 1536), tag=f"av{k}")
                    for k in range(2)
                ]
                t_aoT = [
                    load(hc, d_aoT[128 * k : 128 * (k + 1), :], (128, 1536), tag=f"ao{k}")
                    for k in range(2)
                ]
                t_avb = load(hc, d_avb, (128, 12), F32, tag="avb")
                t_aob = load(hc, d_aob, (128, 12), F32, tag="aob")
                t_clsT = [
                    load(hc, d_clsT[128 * k : 128 * (k + 1), :], (128, 2415), tag=f"cl{k}")
                    for k in range(2)
                ]
                t_clsb = load(hc, d_clsb, (128, 22), F32, tag="clsb")

                prevT = None
                feats = []
                for i in range(6):
                    if i == 0:
                        srcT = t_pooled
                    else:
                        srcT = [hw.tile([128, BC], F32R, tag=f"src{m}", name=f"src{m}") for m in range(2)]
                        for m in range(2):
                            pg = ps_mm.tile([128, HCH], F32, tag="mm", name="mm")
                            for k in range(4):
                                rhs = prevT[k] if k < 2 else t_pooled[k - 2]
                                nc.tensor.matmul(
                                    pg[:, 0:BC],
                                    t_gate[k][
                                        :, DM * (i - 1) + 128 * m : DM * (i - 1) + 128 * (m + 1)
                                    ],
                                    rhs[:, :],
                                    start=(k == 0), stop=(k == 3),
                                )
                            e = hw.tile([128, BC], F32, tag="ge", name="ge")
                            nc.scalar.activation(
                                e[:, :], pg[:, 0:BC], AF.Exp, scale=-1.0,
                                bias=t_gbn[:, 2 * (i - 1) + m : 2 * (i - 1) + m + 1],
                            )
                            f1 = hw.tile([128, BC], F32, tag="gf", name="gf")
                            nc.vector.tensor_scalar(
                                out=f1[:, :], in0=e[:, :], scalar1=1.0, scalar2=None,
                                op0=ALU.add,
                            )
                            gsig = hw.tile([128, BC], F32, tag="gsig", name="gsig")
                            nc.vector.reciprocal(gsig[:, :], f1[:, :])
                            ddif = hw.tile([128, BC], F32, tag="gd", name="gd")
                            nc.vector.tensor_tensor(
                                out=ddif[:, :], in0=prevT[m][:, :].bitcast(F32),
                                in1=t_pooled[m][:, :].bitcast(F32), op=ALU.subtract,
                            )
                            s1 = hw.tile([128, BC], F32, tag="gs1", name="gs1")
                            nc.vector.tensor_tensor(
                                out=s1[:, :], in0=gsig[:, :], in1=ddif[:, :], op=ALU.mult
                            )
                            nc.vector.tensor_tensor(
                                out=srcT[m][:, :], in0=s1[:, :],
                                in1=t_pooled[m][:, :].bitcast(F32), op=ALU.add,
                            )
                    vT = [hw.tile([128, BC], F32R, tag=f"vT{m}", name=f"vT{m}") for m in range(2)]
                    for m in range(2):
                        pv = ps_mm.tile([128, HCH], F32, tag="mm", name="mm")
                        for k in range(2):
                            nc.tensor.matmul(
                                pv[:, 0:BC],
                                t_avT[k][:, DM * i + 128 * m : DM * i + 128 * (m + 1)],
                                srcT[k][:, :],
                                start=(k == 0), stop=(k == 1),
                            )
                        nc.scalar.activation(
                            vT[m][:, :], pv[:, 0:BC], AF.Identity,
                            bias=t_avb[:, 2 * i + m : 2 * i + m + 1],
                        )
                    newprev = [
                        hw.tile([128, BC], F32R, tag=f"ft{i}_{m}", name=f"ft{i}_{m}", bufs=1) for m in range(2)
                    ]
                    for m in range(2):
                        po = ps_mm.tile([128, HCH], F32, tag="mm", name="mm")
                        for k in range(2):
                            nc.tensor.matmul(
                                po[:, 0:BC],
                                t_aoT[k][:, DM * i + 128 * m : DM * i + 128 * (m + 1)],
                                vT[k][:, :],
                                start=(k == 0), stop=(k == 1),
                            )
                        nc.scalar.activation(
                            newprev[m][:, :], po[:, 0:BC], AF.Identity,
                            bias=t_aob[:, 2 * i + m : 2 * i + m + 1],
                        )
                    prevT = newprev
                    feats.append(newprev)

                # classifiers: logitsT = cls_w @ feat + b, then transpose to
                # row-major [b, ncls] and DMA out
                out_rows = [
                    [hw.tile([128, NCLS[i]], F32, tag=f"or{i}_{bb}", name=f"or{i}_{bb}", bufs=1) for bb in range(2)]
                    for i in range(6)
                ]
                for (lvl, moff, msz, bcol) in cls_chunks:
                    pc = ps_mm.tile([128, HCH], F32, tag="mm", name="mm")
                    for k in range(2):
                        nc.tensor.matmul(
                            pc[0:msz, 0:BC],
                            t_clsT[k][:, moff : moff + msz],
                            feats[lvl][k][:, :],
                            start=(k == 0), stop=(k == 1),
                        )
                    logT = hw.tile([128, BC], F32, tag="logT", name="logT")
                    nc.scalar.activation(
                        logT[0:msz, :], pc[0:msz, 0:BC], AF.Identity,
                        bias=t_clsb[:msz, bcol : bcol + 1],
                    )
                    lvl_off = moff - sum(NCLS[:lvl])
                    for bb in range(2):
                        pt = ps_tr.tile([128, 128], F32, tag="mm", name="mm")
                        nc.tensor.transpose(
                            pt[:, 0:msz],
                            logT[0:msz, 128 * bb : 128 * (bb + 1)],
                            t_ident[0:msz, 0:msz].bitcast(F32),
                        )
                        nc.scalar.copy(
                            out_rows[lvl][bb][:, lvl_off : lvl_off + msz],
                            pt[:, 0:msz],
                        )
                for i in range(6):
                    for bb in range(2):
                        nc.sync.dma_start(
                            d_out[i][128 * bb : 128 * (bb + 1), :],
                            out_rows[i][bb][:, :],
                        )
        # end ExitStack
    return nc, din


_CACHE = {}


def _get_program():
    if "prog" not in _CACHE:
        # cls_chunks layout is static
        col = 0
        off = 0
        cls_chunks = []
        for i, n in enumerate(NCLS):
            for mc in range(_cdiv(n, 128)):
                sz = min(128, n - 128 * mc)
                cls_chunks.append((i, off + 128 * mc, sz, col))
                col += 1
            off += n
        nc, din = build_program(cls_chunks)
        _CACHE["prog"] = nc
    return _CACHE["prog"]


def make_in_maps(inputs):
    """Split full inputs into per-core input maps (host-side prep)."""
    w = {
        k: (v if isinstance(v, (tuple, list)) else np.asarray(v, dtype=np.float32))
        for k, v in inputs.items()
    }
    w["cls_w"] = tuple(np.asarray(x, dtype=np.float32) for x in inputs["cls_w"])
    w["cls_b"] = tuple(np.asarray(x, dtype=np.float32) for x in inputs["cls_b"])
    consts, _ = _host_prep(w)
    coi = np.asarray(inputs["coi"], dtype=np.float32)
    mae = np.asarray(inputs["coi_mae"], dtype=np.float32)
    in_maps = []
    for c in range(NCORES):
        m = dict(consts)
        m["coi"] = np.ascontiguousarray(coi[BC * c : BC * (c + 1)])
        m["coi_mae"] = np.ascontiguousarray(mae[BC * c : BC * (c + 1)])
        in_maps.append(m)
    return in_maps


def kernel(**inputs):
    from concourse.bass_utils import run_bass_kernel_spmd

    nc = _get_program()
    in_maps = make_in_maps(inputs)
    res = run_bass_kernel_spmd(nc, in_maps, core_ids=list(range(NCORES)), trace=False)
    outs = []
    for i, n in enumerate(NCLS):
        full = np.concatenate(
            [res.results[c][f"out{i}"] for c in range(NCORES)], axis=0
        )
        outs.append(full)
    return tuple(outs)


# revision 20
# speedup vs baseline: 1.1269x; 1.0892x over previous
"""Trainium2 Bass kernel for the CurriculumDMGHANmae model.

Data-parallel over batch across 8 NeuronCores (256 samples/core). Per core,
the whole network runs in a feature-major layout (features on partitions,
batch*seq on the free dim) so every projection is a PE matmul with no
activations transposed except the initial input transpose.

The Mamba selective scan uses the DVE tensor_tensor_scan instruction: with
partitions = (d, n) state pairs and free = (sample, seqpos), one scan
instruction per 128-partition tile computes the whole recurrence; zeroing
the multiplier at every l==0 column makes the cross-sample scan exact.

Matmuls run in float32r (full-rate PE fp32, ~2^-13 input rounding); the
scan itself and all elementwise math stay in fp32.
"""
import sys

sys.path.insert(0, "/opt/trn_rl_repo")

import numpy as np
import orjson

import concourse.bass as bass
import concourse.bass2jax as bass2jax
import concourse.bass_utils as bass_utils
import concourse.mybir as mybir
import concourse.tile as tile
import bass_rust

F32 = mybir.dt.float32
F32R = mybir.dt.float32r
AF = mybir.ActivationFunctionType
ALU = mybir.AluOpType
AX = mybir.AxisListType

# ---------------------------------------------------------------------------
# Toolchain workarounds (this walrus build rejects >1 sync wait per
# instruction, and the Tile exit drain carries one wait per logical
# processor). Hoist excess waits onto same-engine NoOps at the BIR level.
# ---------------------------------------------------------------------------
_MAXW = 1
_ctr = [0]


def _split_waits_json(bir_bytes):
    d = orjson.loads(bir_bytes)
    changed = False
    for fn in d.get("functions", []):
        for blk in fn.get("blocks", []):
            out = []
            for ins in blk.get("instructions", []):
                si = ins.get("sync_info")
                waits = si.get("on_wait") if si else None
                if waits and len(waits) > _MAXW:
                    extra = waits[:-_MAXW]
                    si["on_wait"] = waits[-_MAXW:]
                    for i in range(0, len(extra), _MAXW):
                        _ctr[0] += 1
                        out.append(
                            {
                                "engine": ins["engine"],
                                "ins": [],
                                "name": f"I-waitsplit-{_ctr[0]}",
                                "opcode": "NoOp",
                                "outs": [],
                                "sync_info": {
                                    "on_update": [],
                                    "on_wait": extra[i : i + _MAXW],
                                },
                            }
                        )
                    changed = True
                out.append(ins)
            blk["instructions"] = out
    return orjson.dumps(d) if changed else bir_bytes


_orig_compile_bir_kernel = bass_utils.compile_bir_kernel


def _patched_compile_bir_kernel(bir_json, tmpdir, neff_name="file.neff"):
    if isinstance(bir_json, str):
        bir_json = bir_json.encode()
    return _orig_compile_bir_kernel(
        _split_waits_json(bir_json), tmpdir, neff_name=neff_name
    )


def _patched_drain_and_barrier(self, tick_clock, wait_clock):
    nc = self.nc
    probe = nc.sync.nop()
    wait_clock.add_sem_waits(
        probe.ins, tile.ScopedClock({None: tick_clock.global_clock})
    )
    si = probe.ins.sync_info
    waits = list(si.on_wait) if si is not None else []
    if len(waits) > 1:
        probe.ins.sync_info = bass_rust.SyncInfo(on_wait=waits[:1], on_update=[])
        for w in waits[1:]:
            extra = nc.sync.nop()
            extra.ins.sync_info = bass_rust.SyncInfo(on_wait=[w], on_update=[])
    nc.sync.drain()
    nc.all_engine_barrier()
    assert self.sems is not None
    popped = nc._tile_sem_poison_stack.pop()
    assert popped is self._sem_poison
    nc.clear_and_free_semaphores(list(self.sems.allocated().values()))
    nc.all_engine_barrier()


def _apply_patches():
    bass_utils.compile_bir_kernel = _patched_compile_bir_kernel
    bass2jax.compile_bir_kernel = _patched_compile_bir_kernel
    tile.TileContext._drain_and_barrier = _patched_drain_and_barrier


_apply_patches()

# ---------------------------------------------------------------------------
# Model constants
# ---------------------------------------------------------------------------
NCORES = 8
B_FULL = 2048
BC = B_FULL // NCORES  # 256 samples per core
L = 12
DM = 256
DI = 512
DS = 16
DTR = 16
NCLS = (5, 30, 80, 200, 600, 1500)
LN_EPS = 1e-5
NQ = 4  # quarters per core
QB = BC // NQ  # 64 samples per quarter
COLS = QB * L  # 768 free columns per quarter
HCH = 384  # psum chunk (half of COLS)


def _cdiv(a, b):
    return (a + b - 1) // b


def _host_prep(w):
    """Build all device-side constant arrays from the raw model weights."""
    f32 = lambda x: np.ascontiguousarray(x, dtype=np.float32)
    WfL = w["fusion_w"][:, :DM]
    WfR = w["fusion_w"][:, DM:]
    out = {}
    out["w1f_T"] = f32((WfL @ w["coi_proj_w"]).T)  # [768, 256]
    out["w2f_T"] = f32((WfR @ w["coi_mae_proj_w"]).T)  # [768, 256]
    bias_fused = WfL @ w["coi_proj_b"] + WfR @ w["coi_mae_proj_b"] + w["fusion_b"]
    out["bias_fused"] = f32(bias_fused.reshape(2, 128).T)  # [128, 2]
    out["ln_g"] = f32(w["ln_g"].reshape(2, 128).T)
    out["ln_b"] = f32(w["ln_b"].reshape(2, 128).T)
    # Wbig: conv folded into the xc half of in_proj.
    Win_x = w["in_proj_w"][:DI]  # [512, 256]
    Win_z = w["in_proj_w"][DI:]  # [512, 256]
    wbig = np.zeros((4 * DM, DI), dtype=np.float32)
    for k in range(4):
        wbig[k * DM : (k + 1) * DM, :] = (w["conv_w"][:, k][:, None] * Win_x).T
    out["wbig_T"] = f32(wbig)  # [1024, 512]
    out["wz_T"] = f32(Win_z.T)  # [256, 512]
    out["conv_b_neg"] = f32(-w["conv_b"].reshape(4, 128).T)  # [128, 4]
    out["conv_b"] = f32(w["conv_b"].reshape(4, 128).T)
    out["x_proj_T"] = f32(w["x_proj_w"].T)  # [512, 48]
    out["dt_proj_T"] = f32(w["dt_proj_w"].T)  # [16, 512]
    out["dt_proj_b"] = f32(w["dt_proj_b"].reshape(4, 128).T)  # [128, 4]
    A = -np.exp(w["A_log"])  # [512, 16]
    acols = np.zeros((128, 64), dtype=np.float32)
    for j in range(64):
        for p in range(128):
            acols[p, j] = A[8 * j + p // 16, p % 16]
    out["a_cols"] = f32(acols)
    out["dp_col"] = f32(w["Dp"].reshape(4, 128).T)  # [128, 4]
    out["out_proj_T"] = f32(w["out_proj_w"].T / float(L))  # [512, 256], pool fold
    selrep = np.zeros((128, 1024), dtype=np.float32)
    for a in range(2):
        for m in range(8):
            for p in range(128):
                selrep[64 * a + 8 * m + p // 16, 128 * m + p] = 1.0
    out["selrep"] = selrep  # [128,1024]: same [64,128] pattern at bases 0/64
    sel16b = np.zeros((48, 128), dtype=np.float32)
    for p in range(128):
        sel16b[16 + p % 16, p] = 1.0
    out["sel16b"] = sel16b  # vs dbcT[0:48]: picks rows 16..31 (B)
    sel16c = np.zeros((48, 128), dtype=np.float32)
    for p in range(128):
        sel16c[32 + p % 16, p] = 1.0
    out["sel16c"] = sel16c  # vs dbcT[0:48]: picks rows 32..47 (C)
    selred = np.zeros((128, 16 * 128), dtype=np.float32)
    for jj in range(16):
        for p in range(128):
            selred[p, 128 * jj + 8 * jj + p // 16] = 1.0
    out["selred"] = f32(selred)  # [128, 2048], slice jj = [:, 128jj:128jj+128]
    out["ones_red"] = f32(np.full((128, 1), 1.0 / DM, dtype=np.float32))
    out["ones_row"] = f32(np.ones((1, 128), dtype=np.float32))
    out["eps_col"] = f32(np.full((128, 1), LN_EPS, dtype=np.float32))
    out["ident"] = f32(np.eye(128, dtype=np.float32))
    # head
    gate_T = np.zeros((DI, 5 * DM), dtype=np.float32)
    for i in range(5):
        gate_T[:, DM * i : DM * (i + 1)] = w["gate_w"][i].T
    out["gate_T"] = f32(gate_T)
    out["gate_b_neg"] = f32(
        -np.stack([w["gate_b"][i].reshape(2, 128).T for i in range(5)], 0)
        .transpose(1, 0, 2)
        .reshape(128, 10)
    )  # col 2i+m
    attn_v_T = np.zeros((DM, 6 * DM), dtype=np.float32)
    attn_o_T = np.zeros((DM, 6 * DM), dtype=np.float32)
    attn_vb = np.zeros((128, 12), dtype=np.float32)
    attn_ob = np.zeros((128, 12), dtype=np.float32)
    for i in range(6):
        wv = w["attn_in_w"][i][2 * DM :]  # [256, 256]
        bv = w["attn_in_b"][i][2 * DM :]
        attn_v_T[:, DM * i : DM * (i + 1)] = wv.T
        attn_o_T[:, DM * i : DM * (i + 1)] = w["attn_out_w"][i].T
        attn_vb[:, 2 * i : 2 * i + 2] = bv.reshape(2, 128).T
        attn_ob[:, 2 * i : 2 * i + 2] = w["attn_out_b"][i].reshape(2, 128).T
    out["attn_v_T"] = f32(attn_v_T)
    out["attn_o_T"] = f32(attn_o_T)
    out["attn_vb"] = f32(attn_vb)
    out["attn_ob"] = f32(attn_ob)
    cls_T = np.zeros((DM, sum(NCLS)), dtype=np.float32)
    off = 0
    cls_chunks = []  # (level, m_off_global, m_size, bias_col)
    cls_b = np.zeros((128, 22), dtype=np.float32)
    col = 0
    for i, n in enumerate(NCLS):
        cls_T[:, off : off + n] = w["cls_w"][i].T
        for mc in range(_cdiv(n, 128)):
            sz = min(128, n - 128 * mc)
            cls_b[:sz, col] = w["cls_b"][i][128 * mc : 128 * mc + sz]
            cls_chunks.append((i, off + 128 * mc, sz, col))
            col += 1
        off += n
    out["cls_T"] = f32(cls_T)
    out["cls_b"] = f32(cls_b)
    return out, cls_chunks


# ---------------------------------------------------------------------------
# Program builder
# ---------------------------------------------------------------------------
def build_program(cls_chunks):
    nc = bass.Bass("TRN2", debug=False, num_devices=NCORES)

    din = {}

    def dram_in(name, shape, dt=F32R):
        din[name] = nc.dram_tensor(name, list(shape), dt, kind="ExternalInput").ap()
        return din[name]

    d_coi = dram_in("coi", (BC, L, 768)).rearrange("b l f -> (b l) f")
    d_mae = dram_in("coi_mae", (BC, 768))
    d_w1f = dram_in("w1f_T", (768, 256))
    d_w2f = dram_in("w2f_T", (768, 256))
    d_biasf = dram_in("bias_fused", (128, 2), F32)
    d_lng = dram_in("ln_g", (128, 2), F32)
    d_lnb = dram_in("ln_b", (128, 2), F32)
    d_wbig = dram_in("wbig_T", (1024, 512))
    d_wz = dram_in("wz_T", (256, 512))
    d_convbn = dram_in("conv_b_neg", (128, 4), F32)
    d_convb = dram_in("conv_b", (128, 4), F32)
    d_xproj = dram_in("x_proj_T", (512, 48))
    d_dtproj = dram_in("dt_proj_T", (16, 512))
    d_dtb = dram_in("dt_proj_b", (128, 4), F32)
    d_acols = dram_in("a_cols", (128, 64), F32)
    d_dp = dram_in("dp_col", (128, 4), F32)
    d_outproj = dram_in("out_proj_T", (512, 256))
    d_selrep = dram_in("selrep", (128, 1024))
    d_sel16b = dram_in("sel16b", (48, 128))
    d_sel16c = dram_in("sel16c", (48, 128))
    d_selred = dram_in("selred", (128, 2048))
    d_onesred = dram_in("ones_red", (128, 1))
    d_onesrow = dram_in("ones_row", (1, 128))
    d_epscol = dram_in("eps_col", (128, 1), F32)
    d_ident = dram_in("ident", (128, 128))
    d_gate = dram_in("gate_T", (512, 1280))
    d_gbn = dram_in("gate_b_neg", (128, 10), F32)
    d_avT = dram_in("attn_v_T", (256, 1536))
    d_aoT = dram_in("attn_o_T", (256, 1536))
    d_avb = dram_in("attn_vb", (128, 12), F32)
    d_aob = dram_in("attn_ob", (128, 12), F32)
    d_clsT = dram_in("cls_T", (256, 2415))
    d_clsb = dram_in("cls_b", (128, 22), F32)

    d_out = [
        nc.dram_tensor(f"out{i}", [BC, n], F32, kind="ExternalOutput").ap()
        for i, n in enumerate(NCLS)
    ]

    with tile.TileContext(nc) as tc:
        from contextlib import ExitStack

        with ExitStack() as ctx:
            cp = ctx.enter_context(tc.tile_pool(name="consts", bufs=1))
            pp = ctx.enter_context(tc.tile_pool(name="persist", bufs=1))

            def load(pool, d, shape, dt=F32R, tag=None):
                t_name = tag or f"c{len(din)}_{id(d)}"
                t = pool.tile(list(shape), dt, tag=t_name, name=t_name)
                nc.sync.dma_start(t[:, :], d)
                return t

            # --- constants ---
            t_w1f = [
                load(cp, d_w1f[128 * k : 128 * (k + 1), :], (128, 256), tag=f"w1f{k}")
                for k in range(6)
            ]
            t_w2f = [
                load(cp, d_w2f[128 * k : 128 * (k + 1), :], (128, 256), tag=f"w2f{k}")
                for k in range(6)
            ]
            t_biasf = load(cp, d_biasf, (128, 2), F32, tag="biasf")
            t_lng = load(cp, d_lng, (128, 2), F32, tag="lng")
            t_lnb = load(cp, d_lnb, (128, 2), F32, tag="lnb")
            t_wbig = [
                load(cp, d_wbig[128 * k : 128 * (k + 1), :], (128, 512), tag=f"wbig{k}")
                for k in range(8)
            ]
            t_wz = [
                load(cp, d_wz[128 * k : 128 * (k + 1), :], (128, 512), tag=f"wz{k}")
                for k in range(2)
            ]
            t_convbn = load(cp, d_convbn, (128, 4), F32, tag="convbn")
            t_convb = load(cp, d_convb, (128, 4), F32, tag="convb")
            t_xproj = [
                load(cp, d_xproj[128 * k : 128 * (k + 1), :], (128, 48), tag=f"xp{k}")
                for k in range(4)
            ]
            t_dtproj = load(cp, d_dtproj, (16, 512), tag="dtp")
            t_dtb = load(cp, d_dtb, (128, 4), F32, tag="dtb")
            t_acols = load(cp, d_acols, (128, 64), F32, tag="acols")
            t_dp = load(cp, d_dp, (128, 4), F32, tag="dp")
            t_outproj = [
                load(cp, d_outproj[128 * k : 128 * (k + 1), :], (128, 256), tag=f"op{k}")
                for k in range(4)
            ]
            t_selrep = load(cp, d_selrep, (128, 1024), tag="selrep")
            t_sel16b = load(cp, d_sel16b, (48, 128), tag="sel16b")
            t_sel16c = load(cp, d_sel16c, (48, 128), tag="sel16c")
            t_selred = load(cp, d_selred, (128, 2048), tag="selred")
            t_onesred = load(cp, d_onesred, (128, 1), tag="onesred")
            t_onesrow = load(cp, d_onesrow, (1, 128), tag="onesrow")
            t_epscol = load(cp, d_epscol, (128, 1), F32, tag="epscol")
            t_ident = load(cp, d_ident, (128, 128), tag="ident")

            # persistent across quarters
            t_pooled = [pp.tile([128, BC], F32R, tag=f"pooled{m}", name=f"pooled{m}") for m in range(2)]
            t_dApp = [
                pp.tile([128, COLS], F32, tag=f"dApp{i}", name=f"dApp{i}")
                for i in range(2)
            ]
            for i in range(2):
                nc.gpsimd.memset(
                    t_dApp[i][:, :].rearrange("p (b l) -> p b l", l=L)[:, :, 0:1], 0.0
                )

            ps_mm = ctx.enter_context(
                tc.tile_pool(name="ps_mm", bufs=2, space="PSUM")
            )
            ps_tr = ps_mm
            ps_strip = ps_mm

            # --- stage 0: mae path ---
            with tc.tile_pool(name="mae", bufs=1) as mp:
                rows = [
                    load(mp, d_mae[128 * r : 128 * (r + 1), :], (128, 768), tag=f"mr{r}")
                    for r in range(2)
                ]
                maeT = [mp.tile([128, 256], F32R, tag=f"maeT{f}", name=f"maeT{f}") for f in range(6)]
                for f in range(6):
                    p = ps_tr.tile([128, 256], F32R, tag="mm", name="mm")
                    for r in range(2):
                        nc.tensor.transpose(
                            p[:, 128 * r : 128 * (r + 1)],
                            rows[r][:, 128 * f : 128 * (f + 1)],
                            t_ident[:, :],
                        )
                    nc.scalar.copy(maeT[f][:, :], p[:, :].bitcast(F32))
                t_maef = [
                    pp.tile([128, BC], F32, tag=f"maef{m}", name=f"maef{m}") for m in range(2)
                ]
                for m in range(2):
                    p = ps_mm.tile([128, 384], F32, tag="mm", name="mm")
                    for k in range(6):
                        nc.tensor.matmul(
                            p[:, 0:256],
                            t_w2f[k][:, 128 * m : 128 * (m + 1)],
                            maeT[k][:, :],
                            start=(k == 0),
                            stop=(k == 5),
                        )
                    nc.scalar.activation(
                        t_maef[m][:, :], p[:, 0:256], AF.Identity,
                        bias=t_biasf[:, m : m + 1],
                    )

            # --- quarters ---
            qctx = ExitStack()
            wp = qctx.enter_context(tc.tile_pool(name="work", bufs=1))
            wp2 = qctx.enter_context(tc.tile_pool(name="work2", bufs=2))
            sp = qctx.enter_context(tc.tile_pool(name="scan", bufs=2))
            ps_a = qctx.enter_context(tc.tile_pool(name="ps_a", bufs=2, space="PSUM"))
            ps_y = qctx.enter_context(tc.tile_pool(name="ps_y", bufs=1, space="PSUM"))

            for q in range(NQ):
                # Phase A: load + transpose coi
                coiT = [wp2.tile([128, COLS], F32R, tag=f"coiT{f}", name=f"coiT{f}", bufs=1) for f in range(6)]
                for r in range(6):
                    rt = wp2.tile([128, 768], F32R, tag="coirow", name="coirow", bufs=1)
                    # rows of this tile are (l, b) l-major: 2 l values x 64 b
                    for i in range(2):
                        nc.sync.dma_start(
                            rt[64 * i : 64 * (i + 1), :],
                            d_coi.rearrange("(b l) f -> b l f", l=L)[
                                QB * q : QB * (q + 1), 2 * r + i, :
                            ],
                        )
                    for fp in range(3):
                        p = ps_tr.tile([128, 256], F32R, tag="mm", name="mm")
                        for f2 in range(2):
                            f = 2 * fp + f2
                            nc.tensor.transpose(
                                p[:, 128 * f2 : 128 * (f2 + 1)],
                                rt[:, 128 * f : 128 * (f + 1)],
                                t_ident[:, :],
                            )
                        for f2 in range(2):
                            f = 2 * fp + f2
                            nc.scalar.copy(
                                coiT[f][:, 128 * r : 128 * (r + 1)],
                                p[:, 128 * f2 : 128 * (f2 + 1)].bitcast(F32),
                            )

                # Phase B: fused projection + mae + bias + relu
                fusedT = [wp.tile([128, COLS], F32R, tag=f"fusedT{m}", name=f"fusedT{m}") for m in range(2)]
                for m in range(2):
                    for c in range(2):
                        p = ps_mm.tile([128, HCH], F32, tag="mm", name="mm")
                        for k in range(6):
                            nc.tensor.matmul(
                                p[:, :],
                                t_w1f[k][:, 128 * m : 128 * (m + 1)],
                                coiT[k][:, HCH * c : HCH * (c + 1)],
                                start=(k == 0),
                                stop=(k == 5),
                            )
                        tmp = wp2.tile([128, HCH], F32, tag="y1", name="btmp", bufs=1)
                        nc.vector.scalar_tensor_tensor(
                            out=tmp[:, :].rearrange("p (l b) -> p b l", b=QB),
                            in0=p[:, :].rearrange("p (l b) -> p b l", b=QB),
                            scalar=t_biasf[:, m : m + 1],
                            in1=t_maef[m][:, QB * q : QB * (q + 1)]
                            .broadcast_to([128, QB, 6]),
                            op0=ALU.add,
                            op1=ALU.add,
                        )
                        nc.scalar.activation(
                            fusedT[m][:, HCH * c : HCH * (c + 1)], tmp[:, :], AF.Relu
                        )

                # Phase C: LayerNorm
                lnT = [wp2.tile([128, COLS], F32R, tag=f"coiT{m}", name=f"lnT{m}", bufs=1) for m in range(2)]
                sqT = [wp2.tile([128, COLS], F32R, tag=f"sqT{m}", name=f"sqT{m}", bufs=1) for m in range(2)]
                for m in range(2):
                    nc.scalar.activation(sqT[m][:, :], fusedT[m][:, :].bitcast(F32), AF.Square)
                t2T = [wp2.tile([128, COLS], F32, tag=f"t2T{m}", name=f"t2T{m}", bufs=1) for m in range(2)]
                reps = []
                for c in range(2):
                    pmean = ps_strip.tile([1, HCH], F32, tag="mm", name="pmean")
                    pex2 = ps_strip.tile([1, HCH], F32, tag="mm", name="pex2")
                    for m in range(2):
                        nc.tensor.matmul(
                            pmean[:, :], t_onesred[:, :],
                            fusedT[m][:, HCH * c : HCH * (c + 1)],
                            start=(m == 0), stop=(m == 1),
                        )
                        nc.tensor.matmul(
                            pex2[:, :], t_onesred[:, :],
                            sqT[m][:, HCH * c : HCH * (c + 1)],
                            start=(m == 0), stop=(m == 1),
                        )
                    mean_sb = wp2.tile([1, HCH], F32R, tag="mean_sb", name="mean_sb", bufs=2)
                    nc.scalar.copy(mean_sb[:, :], pmean[:, :])
                    msq = wp2.tile([1, HCH], F32, tag="msq", name="msq")
                    nc.vector.tensor_tensor(
                        out=msq[:, :], in0=mean_sb[:, :].bitcast(F32),
                        in1=mean_sb[:, :].bitcast(F32), op=ALU.mult,
                    )
                    var = wp2.tile([1, HCH], F32, tag="var", name="var")
                    nc.vector.tensor_tensor(
                        out=var[:, :], in0=pex2[:, :], in1=msq[:, :], op=ALU.subtract,
                    )
                    lnv = wp2.tile([1, HCH], F32, tag="lnv", name="lnv")
                    nc.scalar.activation(
                        lnv[:, :], var[:, :], AF.Ln, bias=t_epscol[0:1, :]
                    )
                    rstd = wp2.tile([1, HCH], F32R, tag="rstd", name="rstd", bufs=2)
                    nc.scalar.activation(rstd[:, :], lnv[:, :], AF.Exp, scale=-0.5)
                    reps.append((mean_sb, rstd))
                for c in range(2):
                    mean_sb, rstd = reps[c]
                    prep_m = ps_strip.tile([128, HCH], F32, tag="mm", name="prepm")
                    nc.tensor.matmul(
                        prep_m[:, :], t_onesrow[:, :], mean_sb[:, :],
                        start=True, stop=True,
                    )
                    prep_r = ps_strip.tile([128, HCH], F32, tag="mm", name="prepr")
                    nc.tensor.matmul(
                        prep_r[:, :], t_onesrow[:, :], rstd[:, :],
                        start=True, stop=True,
                    )
                    for m in range(2):
                        t1 = wp2.tile([128, HCH], F32, tag="lnt1", name="lnt1", bufs=1)
                        nc.vector.tensor_tensor(
                            out=t1[:, :],
                            in0=fusedT[m][:, HCH * c : HCH * (c + 1)].bitcast(F32),
                            in1=prep_m[:, :], op=ALU.subtract,
                        )
                        nc.vector.tensor_tensor(
                            out=t2T[m][:, HCH * c : HCH * (c + 1)],
                            in0=t1[:, :], in1=prep_r[:, :], op=ALU.mult,
                        )
                for m in range(2):
                    nc.scalar.activation(
                        lnT[m][:, :], t2T[m][:, :], AF.Identity,
                        bias=t_lnb[:, m : m + 1], scale=t_lng[:, m : m + 1],
                    )

                # Phase D: in_proj with folded conv (xc) and z + silu
                xcT = [wp.tile([128, COLS], F32R, tag=f"xcT{g}", name=f"xcT{g}") for g in range(4)]
                szT = [wp.tile([128, COLS], F32, tag=f"szT{g}", name=f"szT{g}") for g in range(4)]
                for g in range(4):
                    for c in range(2):
                        p = ps_mm.tile([128, HCH], F32, tag="mm", name="mm")
                        # shift k=3 (offset 0), full range, starts accumulation
                        for h in range(2):
                            nc.tensor.matmul(
                                p[:, :],
                                t_wbig[6 + h][:, 128 * g : 128 * (g + 1)],
                                lnT[h][:, HCH * c : HCH * (c + 1)],
                                start=(h == 0), stop=False,
                                skip_group_check=True,
                            )
                        for k in (2, 1, 0):
                            o = 3 - k
                            for h in range(2):
                                if c == 0:
                                    outap = p[:, QB * o : HCH]
                                    rhsap = lnT[h][:, 0 : HCH - QB * o]
                                else:
                                    outap = p[:, :]
                                    rhsap = lnT[h][:, HCH - QB * o : 2 * HCH - QB * o]
                                nc.tensor.matmul(
                                    outap,
                                    t_wbig[2 * k + h][:, 128 * g : 128 * (g + 1)],
                                    rhsap,
                                    start=False, stop=(k == 0 and h == 1),
                                    skip_group_check=True,
                                )
                        # silu evac: xc = (p + b) * sigmoid(p + b)
                        e = wp2.tile([128, HCH], F32, tag="se", name="se", bufs=1)
                        nc.scalar.activation(
                            e[:, :], p[:, :], AF.Exp, scale=-1.0,
                            bias=t_convbn[:, g : g + 1],
                        )
                        f1 = wp2.tile([128, HCH], F32, tag="sf", name="sf", bufs=1)
                        nc.vector.tensor_scalar(
                            out=f1[:, :], in0=e[:, :], scalar1=1.0, scalar2=None,
                            op0=ALU.add,
                        )
                        r1 = wp2.tile([128, HCH], F32, tag="sr", name="sr", bufs=1)
                        nc.vector.reciprocal(r1[:, :], f1[:, :])
                        nc.vector.scalar_tensor_tensor(
                            out=xcT[g][:, HCH * c : HCH * (c + 1)],
                            in0=p[:, :], scalar=t_convb[:, g : g + 1],
                            in1=r1[:, :], op0=ALU.add, op1=ALU.mult,
                        )
                for g in range(4):
                    for c in range(2):
                        p = ps_mm.tile([128, HCH], F32, tag="mm", name="mm")
                        for h in range(2):
                            nc.tensor.matmul(
                                p[:, :],
                                t_wz[h][:, 128 * g : 128 * (g + 1)],
                                lnT[h][:, HCH * c : HCH * (c + 1)],
                                start=(h == 0), stop=(h == 1),
                            )
                        e = wp2.tile([128, HCH], F32, tag="se", name="se", bufs=1)
                        nc.scalar.activation(e[:, :], p[:, :], AF.Exp, scale=-1.0)
                        f1 = wp2.tile([128, HCH], F32, tag="sf", name="sf", bufs=1)
                        nc.vector.tensor_scalar(
                            out=f1[:, :], in0=e[:, :], scalar1=1.0, scalar2=None,
                            op0=ALU.add,
                        )
                        r1 = wp2.tile([128, HCH], F32, tag="sr", name="sr", bufs=1)
                        nc.vector.reciprocal(r1[:, :], f1[:, :])
                        nc.vector.scalar_tensor_tensor(
                            out=szT[g][:, HCH * c : HCH * (c + 1)],
                            in0=p[:, :], scalar=0.0,
                            in1=r1[:, :], op0=ALU.bypass, op1=ALU.mult,
                        )

                # Phase E: x_proj -> dbc; dt; u; B/C replication
                dbcT = wp.tile([48, COLS], F32R, tag="dbcT", name="dbcT")
                for c in range(2):
                    p = ps_mm.tile([128, HCH], F32, tag="mm", name="mm")
                    for k in range(4):
                        nc.tensor.matmul(
                            p[0:48, :], t_xproj[k][:, :],
                            xcT[k][:, HCH * c : HCH * (c + 1)],
                            start=(k == 0), stop=(k == 3),
                        )
                    nc.scalar.copy(
                        dbcT[:, HCH * c : HCH * (c + 1)], p[0:48, :]
                    )
                dtT = [wp.tile([128, COLS], F32R, tag=f"dtT{g}", name=f"dtT{g}") for g in range(4)]
                for g in range(4):
                    for c in range(2):
                        p = ps_mm.tile([128, HCH], F32, tag="mm", name="mm")
                        nc.tensor.matmul(
                            p[:, :], t_dtproj[:, 128 * g : 128 * (g + 1)],
                            dbcT[0:16, HCH * c : HCH * (c + 1)],
                            start=True, stop=True,
                        )
                        e = wp2.tile([128, HCH], F32, tag="spe", name="spe", bufs=1)
                        nc.scalar.activation(
                            e[:, :], p[:, :], AF.Exp, bias=t_dtb[:, g : g + 1]
                        )
                        nc.scalar.activation(
                            dtT[g][:, HCH * c : HCH * (c + 1)], e[:, :], AF.Ln,
                            bias=1.0,
                        )
                uT = [wp.tile([128, COLS], F32R, tag=f"uT{g}", name=f"uT{g}") for g in range(4)]
                for g in range(4):
                    nc.vector.tensor_tensor(
                        out=uT[g][:, :], in0=dtT[g][:, :].bitcast(F32),
                        in1=xcT[g][:, :].bitcast(F32), op=ALU.mult,
                    )
                t_brep = wp.tile([128, COLS], F32, tag="brep", name="brep")
                t_crep = wp.tile([128, COLS], F32, tag="crep", name="crep")
                for c in range(2):
                    pb = ps_mm.tile([128, HCH], F32, tag="mm", name="mm")
                    nc.tensor.matmul(
                        pb[:, :], t_sel16b[:, :], dbcT[0:48, HCH * c : HCH * (c + 1)],
                        start=True, stop=True,
                    )
                    nc.scalar.activation(
                        t_brep[:, :].rearrange("p (b l) -> p b l", l=L)[
                            :, :, 6 * c : 6 * (c + 1)
                        ],
                        pb[:, :].rearrange("p (l b) -> p b l", b=QB),
                        AF.Copy,
                    )
                    pc = ps_mm.tile([128, HCH], F32, tag="mm", name="mm")
                    nc.tensor.matmul(
                        pc[:, :], t_sel16c[:, :], dbcT[0:48, HCH * c : HCH * (c + 1)],
                        start=True, stop=True,
                    )
                    nc.scalar.activation(
                        t_crep[:, :].rearrange("p (b l) -> p b l", l=L)[
                            :, :, 6 * c : 6 * (c + 1)
                        ],
                        pc[:, :].rearrange("p (l b) -> p b l", b=QB),
                        AF.Copy,
                    )

                # Phase F: scan core over 64 dn-tiles
                y2T = [wp.tile([128, COLS], F32R, tag=(f"fusedT{g}" if g < 2 else f"y2T{g}"), name=f"y2T{g}") for g in range(4)]
                for g in range(4):
                    py = ps_y.tile([128, COLS], F32, tag="py", name="py")
                    for jj in range(16):
                        j = 16 * g + jj
                        prow = 8 * jj
                        ja, jm = jj // 8, jj % 8
                        lsel = t_selrep[64 * ja : 64 * (ja + 1), 128 * jm : 128 * (jm + 1)]
                        pa = ps_a.tile([128, COLS], F32, tag="pab", name="pab")
                        nc.tensor.matmul(
                            pa[:, 0:512], lsel,
                            dtT[g][64 * ja : 64 * (ja + 1), 0:512],
                            start=True, stop=True,
                        )
                        nc.tensor.matmul(
                            pa[:, 512:768], lsel,
                            dtT[g][64 * ja : 64 * (ja + 1), 512:768],
                            start=True, stop=True,
                        )
                        dA = t_dApp[(64 * q + j) % 2]
                        nc.scalar.activation(
                            dA[:, :].rearrange("p (b l) -> p b l", l=L)[:, :, 1:L],
                            pa[:, :].rearrange("p (l b) -> p b l", b=QB)[:, :, 1:L],
                            AF.Exp, scale=t_acols[:, j : j + 1],
                        )
                        pb = ps_a.tile([128, COLS], F32, tag="pab", name="pab")
                        nc.tensor.matmul(
                            pb[:, 0:512], lsel,
                            uT[g][64 * ja : 64 * (ja + 1), 0:512],
                            start=True, stop=True,
                        )
                        nc.tensor.matmul(
                            pb[:, 512:768], lsel,
                            uT[g][64 * ja : 64 * (ja + 1), 512:768],
                            start=True, stop=True,
                        )
                        dBx = sp.tile([128, COLS], F32, tag="dBx", name="dBx")
                        nc.vector.tensor_tensor(
                            out=dBx[:, :].rearrange("p (b l) -> p b l", l=L),
                            in0=pb[:, :].rearrange("p (l b) -> p b l", b=QB),
                            in1=t_brep[:, :].rearrange("p (b l) -> p b l", l=L),
                            op=ALU.mult,
                        )
                        H = sp.tile([128, COLS], F32, tag="H", name="H")
                        nc.vector.tensor_tensor_scan(
                            H[:, :], dA[:, :], dBx[:, :], 0.0, ALU.mult, ALU.add
                        )
                        Hc = sp.tile([128, COLS], F32R, tag="Hc", name="Hc", bufs=2)
                        nc.gpsimd.tensor_tensor(
                            out=Hc[:, :], in0=H[:, :], in1=t_crep[:, :], op=ALU.mult
                        )
                        nc.tensor.matmul(
                            py[:, 0:512], t_selred[:, 128 * jj : 128 * (jj + 1)],
                            Hc[:, 0:512], start=(jj == 0), stop=(jj == 15),
                            skip_group_check=True,
                        )
                        nc.tensor.matmul(
                            py[:, 512:768], t_selred[:, 128 * jj : 128 * (jj + 1)],
                            Hc[:, 512:768], start=(jj == 0), stop=(jj == 15),
                            skip_group_check=True,
                        )
                    for c in range(2):
                        y1 = wp2.tile([128, HCH], F32, tag="y1", name="y1", bufs=1)
                        nc.vector.scalar_tensor_tensor(
                            out=y1[:, :].rearrange("p (b l) -> p b l", l=L),
                            in0=xcT[g][:, :].bitcast(F32).rearrange(
                                "p (l b) -> p b l", b=QB
                            )[:, 32 * c : 32 * (c + 1), :],
                            scalar=t_dp[:, g : g + 1],
                            in1=py[:, HCH * c : HCH * (c + 1)].rearrange(
                                "p (b l) -> p b l", l=L
                            ),
                            op0=ALU.mult, op1=ALU.add,
                        )
                        nc.vector.tensor_tensor(
                            out=y2T[g][:, HCH * c : HCH * (c + 1)].rearrange(
                                "p (b l) -> p b l", l=L
                            ),
                            in0=y1[:, :].rearrange("p (b l) -> p b l", l=L),
                            in1=szT[g][:, :].rearrange(
                                "p (l b) -> p b l", b=QB
                            )[:, 32 * c : 32 * (c + 1), :],
                            op=ALU.mult,
                        )

                # Phase G: out_proj (scaled by 1/L) + pooling
                for m in range(2):
                    for c in range(2):
                        p = ps_mm.tile([128, HCH], F32, tag="mm", name="mm")
                        for k in range(4):
                            nc.tensor.matmul(
                                p[:, :], t_outproj[k][:, 128 * m : 128 * (m + 1)],
                                y2T[k][:, HCH * c : HCH * (c + 1)],
                                start=(k == 0), stop=(k == 3),
                            )
                        with nc.allow_low_precision(reason="f32r out is fp32 bits"):
                            nc.vector.reduce_sum(
                                t_pooled[m][:, QB * q + 32 * c : QB * q + 32 * (c + 1)],
                                p[:, :].rearrange("p (b l) -> p b l", l=L),
                                axis=AX.X,
                            )

            qctx.close()

            # --- head ---
            with tc.tile_pool(name="headc", bufs=1) as hc, tc.tile_pool(
                name="headw", bufs=2
            ) as hw:
                t_gate = [
                    load(hc, d_gate[128 * k : 128 * (k + 1), :], (128, 1280), tag=f"g{k}")
                    for k in range(4)
                ]
                t_gbn = load(hc, d_gbn, (128, 10), F32, tag="gbn")
                t_avT = [
                    load(hc, d_avT[128 * k : 128 * (k + 1), :], (128, 1536), tag=f"av{k}")
                    for k in range(2)
                ]
                t_aoT = [
                    load(hc, d_aoT[128 * k : 128 * (k + 1), :], (128, 1536), tag=f"ao{k}")
                    for k in range(2)
                ]
                t_avb = load(hc, d_avb, (128, 12), F32, tag="avb")
                t_aob = load(hc, d_aob, (128, 12), F32, tag="aob")
                t_clsT = [
                    load(hc, d_clsT[128 * k : 128 * (k + 1), :], (128, 2415), tag=f"cl{k}")
                    for k in range(2)
                ]
                t_clsb = load(hc, d_clsb, (128, 22), F32, tag="clsb")

                prevT = None
                feats = []
                for i in range(6):
                    if i == 0:
                        srcT = t_pooled
                    else:
                        srcT = [hw.tile([128, BC], F32R, tag=f"src{m}", name=f"src{m}") for m in range(2)]
                        for m in range(2):
                            pg = ps_mm.tile([128, HCH], F32, tag="mm", name="mm")
                            for k in range(4):
                                rhs = prevT[k] if k < 2 else t_pooled[k - 2]
                                nc.tensor.matmul(
                                    pg[:, 0:BC],
                                    t_gate[k][
                                        :, DM * (i - 1) + 128 * m : DM * (i - 1) + 128 * (m + 1)
                                    ],
                                    rhs[:, :],
                                    start=(k == 0), stop=(k == 3),
                                )
                            e = hw.tile([128, BC], F32, tag="ge", name="ge")
                            nc.scalar.activation(
                                e[:, :], pg[:, 0:BC], AF.Exp, scale=-1.0,
                                bias=t_gbn[:, 2 * (i - 1) + m : 2 * (i - 1) + m + 1],
                            )
                            f1 = hw.tile([128, BC], F32, tag="gf", name="gf")
                            nc.vector.tensor_scalar(
                                out=f1[:, :], in0=e[:, :], scalar1=1.0, scalar2=None,
                                op0=ALU.add,
                            )
                            gsig = hw.tile([128, BC], F32, tag="gsig", name="gsig")
                            nc.vector.reciprocal(gsig[:, :], f1[:, :])
                            ddif = hw.tile([128, BC], F32, tag="gd", name="gd")
                            nc.vector.tensor_tensor(
                                out=ddif[:, :], in0=prevT[m][:, :].bitcast(F32),
                                in1=t_pooled[m][:, :].bitcast(F32), op=ALU.subtract,
                            )
                            s1 = hw.tile([128, BC], F32, tag="gs1", name="gs1")
                            nc.vector.tensor_tensor(
                                out=s1[:, :], in0=gsig[:, :], in1=ddif[:, :], op=ALU.mult
                            )
                            nc.vector.tensor_tensor(
                                out=srcT[m][:, :], in0=s1[:, :],
                                in1=t_pooled[m][:, :].bitcast(F32), op=ALU.add,
                            )
                    vT = [hw.tile([128, BC], F32R, tag=f"vT{m}", name=f"vT{m}") for m in range(2)]
                    for m in range(2):
                        pv = ps_mm.tile([128, HCH], F32, tag="mm", name="mm")
                        for k in range(2):
                            nc.tensor.matmul(
                                pv[:, 0:BC],
                                t_avT[k][:, DM * i + 128 * m : DM * i + 128 * (m + 1)],
                                srcT[k][:, :],
                                start=(k == 0), stop=(k == 1),
                            )
                        nc.scalar.activation(
                            vT[m][:, :], pv[:, 0:BC], AF.Identity,
                            bias=t_avb[:, 2 * i + m : 2 * i + m + 1],
                        )
                    newprev = [
                        hw.tile([128, BC], F32R, tag=f"ft{i}_{m}", name=f"ft{i}_{m}", bufs=1) for m in range(2)
                    ]
                    for m in range(2):
                        po = ps_mm.tile([128, HCH], F32, tag="mm", name="mm")
                        for k in range(2):
                            nc.tensor.matmul(
                                po[:, 0:BC],
                                t_aoT[k][:, DM * i + 128 * m : DM * i + 128 * (m + 1)],
                                vT[k][:, :],
                                start=(k == 0), stop=(k == 1),
                            )
                        nc.scalar.activation(
                            newprev[m][:, :], po[:, 0:BC], AF.Identity,
                            bias=t_aob[:, 2 * i + m : 2 * i + m + 1],
                        )
                    prevT = newprev
                    feats.append(newprev)

                # classifiers: logitsT = cls_w @ feat + b, then transpose to
                # row-major [b, ncls] and DMA out
                out_rows = [
                    [hw.tile([128, NCLS[i]], F32, tag=f"or{i}_{bb}", name=f"or{i}_{bb}", bufs=1) for bb in range(2)]
                    for i in range(6)
                ]
                for (lvl, moff, msz, bcol) in cls_chunks:
                    pc = ps_mm.tile([128, HCH], F32, tag="mm", name="mm")
                    for k in range(2):
                        nc.tensor.matmul(
                            pc[0:msz, 0:BC],
                            t_clsT[k][:, moff : moff + msz],
                            feats[lvl][k][:, :],
                            start=(k == 0), stop=(k == 1),
                        )
                    logT = hw.tile([128, BC], F32, tag="logT", name="logT")
                    nc.scalar.activation(
                        logT[0:msz, :], pc[0:msz, 0:BC], AF.Identity,
                        bias=t_clsb[:msz, bcol : bcol + 1],
                    )
                    lvl_off = moff - sum(NCLS[:lvl])
                    for bb in range(2):
                        pt = ps_tr.tile([128, 128], F32, tag="mm", name="mm")
                        nc.tensor.transpose(
                            pt[:, 0:msz],
                            logT[0:msz, 128 * bb : 128 * (bb + 1)],
                            t_ident[0:msz, 0:msz].bitcast(F32),
                        )
                        nc.scalar.copy(
                            out_rows[lvl][bb][:, lvl_off : lvl_off + msz],
                            pt[:, 0:msz],
                        )
                for i in range(6):
                    for bb in range(2):
                        nc.sync.dma_start(
                            d_out[i][128 * bb : 128 * (bb + 1), :],
                            out_rows[i][bb][:, :],
                        )
        # end ExitStack
    return nc, din


_CACHE = {}


def _get_program():
    if "prog" not in _CACHE:
        # cls_chunks layout is static
        col = 0
        off = 0
        cls_chunks = []
        for i, n in enumerate(NCLS):
            for mc in range(_cdiv(n, 128)):
                sz = min(128, n - 128 * mc)
                cls_chunks.append((i, off + 128 * mc, sz, col))
                col += 1
            off += n
        nc, din = build_program(cls_chunks)
        _CACHE["prog"] = nc
    return _CACHE["prog"]


def make_in_maps(inputs):
    """Split full inputs into per-core input maps (host-side prep)."""
    w = {
        k: (v if isinstance(v, (tuple, list)) else np.asarray(v, dtype=np.float32))
        for k, v in inputs.items()
    }
    w["cls_w"] = tuple(np.asarray(x, dtype=np.float32) for x in inputs["cls_w"])
    w["cls_b"] = tuple(np.asarray(x, dtype=np.float32) for x in inputs["cls_b"])
    consts, _ = _host_prep(w)
    coi = np.asarray(inputs["coi"], dtype=np.float32)
    mae = np.asarray(inputs["coi_mae"], dtype=np.float32)
    in_maps = []
    for c in range(NCORES):
        m = dict(consts)
        m["coi"] = np.ascontiguousarray(coi[BC * c : BC * (c + 1)])
        m["coi_mae"] = np.ascontiguousarray(mae[BC * c : BC * (c + 1)])
        in_maps.append(m)
    return in_maps


def kernel(**inputs):
    from concourse.bass_utils import run_bass_kernel_spmd

    nc = _get_program()
    in_maps = make_in_maps(inputs)
    res = run_bass_kernel_spmd(nc, in_maps, core_ids=list(range(NCORES)), trace=False)
    outs = []
    for i, n in enumerate(NCLS):
        full = np.concatenate(
            [res.results[c][f"out{i}"] for c in range(NCORES)], axis=0
        )
        outs.append(full)
    return tuple(outs)
